# revision 34
# baseline (speedup 1.0000x reference)
"""Bundle-adjustment forward projection on 8 Trainium2 NeuronCores.

reference:  R = euler_to_matrix(euler_angles)            [V,3,3]
            pc = einsum('nj,vij->vni', points3d, R) + t  [V,N,3]
            Zc = min(pc_z, -1e-4)
            u = -f*Xc/Zc + CX ; v = f*Yc/Zc + CY         -> [V,N,2]

FAST PATH ("poly"): the projective division is absorbed into the matmul.
With a = -tz > 0 and y = (r2.p)/a bounded (|y| <= ~0.25 for this
problem's geometry), 1/znega = (1/a)(1 + y + y^2 + y^3) + O(y^4), so the
quantized output q_u = (u - CX)/s_u = (f/(s_u a))(r0.p + tx)(1+y+y^2+y^3)
is a polynomial in the point coordinates, truncated to the K=20 monomials
of degree <= 3 (the dropped deg-4 part is < 1.2 px worst-case):
q_u[v,n] = sum_m W_u[m,v] * mu_m(p_n).  The device then does NO
reciprocals and NO elementwise multiplies -- just two K=20 bf16 matmul
streams (u on PE row strip 0 via tile_position (0,0), v on strip 2 via
(64,0), overlapping) plus exactly one PSUM f32 -> SBUF int8 cast per
output element.  Total error ~2.3 px on a ~778 px scale (rel 3.0e-3)
vs the 2e-2 gate.

Poly device pipeline per 512-point chunk: 2 matmuls (u, v) into a
[128,1024] 2-bank PSUM tile (bufs=4 = all 8 banks -- small tiles keep
matmul latency OFF the cast critical path), one FD=1024 cast
(ScalarE activation-Copy or VectorE tensor_copy, both 1x rate on PSUM
f32 src, split 26/23 by measured speed and running CONCURRENTLY on
different banks), one 256 KB int8 store per chunk pair.  The cast pair
is the wall: both engines run ~95% dense for ~28 us.  N is padded
25000 -> 49*512 = 25088 per core.

Measured phase budget (of ~44 us): ~7.5 us fixed NEFF/framework
preamble, ~2.5 us first-input DMA round trip (weights ride in the same
blob as chunk 0, descriptor gen + ~2 us completion receipt), ~29 us
cast-bound steady state, ~4 us tail (last store receipt + drains; the
cast schedule ends on the faster ScalarE).  Things that DON'T work:
loads on the scalar ring (ACT's FIFO stalls ~1.3 us per skinny DMA
trigger and the casts starve); all stores on one ring (~0.85 us/store
ring time saturates it); late stores on the gpsimd ring (the SWDGE
end-drain waits ~3.5 us on its own last receipt); N=1024 matmuls
(2-bank PSUM output -- builds in bass but the NEFF compiler rejects
it); splitting the final chunk's cast/store across engines (regressed).

The legacy reciprocal-based kernel below is kept as a fallback for
inputs where the series gates fail (z-clamp possible, |y| too large, or
int8 scale too coarse).  Its design notes follow:

The kernel quantizes the output: u = CX + s_u[view] * q where q is stored
as int8 and s_u is a per-view scale folded into the matmul weights on the
host (CX = img/2 = 512 cancels exactly: q = (f/s_u)*Xc * (1/znega)).
The worst-case quantization error is ~2 px on a ~770 px scale (~5e-3
relative, gate is 2e-2).  This cuts the HBM store to 6.4 MB/core.  If
the host-side bound makes the scale too coarse (or the Z clamp can
fire), it falls back to an fp16 variant.

Sharding: N=200000 points split across 8 cores (25000 each); every core
computes all V=128 views so the SBUF partition dim is fully used.

PE: u, v, z matmuls live on three separate 32-row groups of the PE array
(tile_position rows 0/32/64) so the three 500-column streams run
concurrently.  Each row group has its own copy of the point columns in
SBUF partitions 0-10 / 32-42 / 64-74.

Pipeline per 500-point chunk: z matmul -> per-chunk ACT Reciprocal into
a deep ring -> u,v matmuls into a 3-buf PSUM ring -> one DVE broadcast
tensor_tensor computing both quotients (fp32 PSUM x recip -> int8 SBUF)
-> per-pair 256 KB store.  Chunk e=1 of every other mid-stream pair is
computed by the GPSIMD engine instead (ACT stages U,V to SBUF bf16, Q7
multiplies into its own ring and stores fp16 via the SWDGE queue),
taking ~11 us off the DVE critical path.

Hard-won scheduling facts baked in here: (1) deep rings everywhere --
compute must never wait on a store's HBM-receipt semaphore (~3.5 us) or
on the 8 DMAHW completion lanes, so the output ring is 12 pairs deep;
(2) input rides TWO DGE paths in parallel (z+v sections on the gpsimd
SWDGE queue, u section on the scalar HWDGE ring which is otherwise
unused) -- a single queue serializes the 1.66 MB input behind ~93 GB/s
and starves the early pipeline; (3) the offloaded chunks' fp16 halves
store from the gpsimd queue itself so the sync store FIFO never waits
on the late Q7 finish.  Engines measure ~47/48/43/38 us busy
(DVE/ACT/PE/GPSIMD) over a ~66 us kernel -- the DVE stream and the ACT
recip+staging stream are the twin walls.

Numerics: inputs ship as a bf16 hi/lo split (K=11 rows against weight
columns [w_hi, w_hi, w_lo, b_hi, b_lo]); only w_lo*p_lo (~2^-18) is
dropped.

Walrus in this build accepts at most ONE semaphore wait per instruction:
TileContext's tail drain is patched to split its waits into one-wait
nops, and a serialized-BIR rewriter injects same-engine NoOps for any
remaining multi-wait instruction.
"""

import numpy as np
from contextlib import ExitStack

import concourse.bass as bass
import concourse.tile as tile
from concourse import mybir
from concourse.bass_utils import run_bass_kernel_spmd
from concourse.vector_clock import ScopedClock, VectorClock

CX = 512.0
CY = 512.0
Z_MAX = -1e-4

N_CORES = 8
N_POINTS = 200000
N_VIEWS = 128
NPC = N_POINTS // N_CORES          # 25000 points per core
CHUNK = 500                        # matmul free dim (fits one PSUM bank)
CHUNKS = NPC // CHUNK              # 50
PAIRS = CHUNKS // 2                # 25
WCOLS = N_VIEWS                    # weight columns per blob section
BLOB = WCOLS + NPC                 # 25128 cols per 11-row section
KROWS = 11
BANK = 512

# chunk e=1 of these pairs runs on the GPSIMD engine instead of the DVE
OFF_PAIRS = (2, 4, 6, 8, 10, 12, 14, 16, 18, 20)
OFF_IDX = {p: i for i, p in enumerate(OFF_PAIRS)}

F32 = mybir.dt.float32
F16 = mybir.dt.float16
I8 = mybir.dt.int8
BF16 = mybir.dt.bfloat16


# ---------------------------------------------------------------------------
# Tile tail-drain workaround: this walrus build only accepts ONE semaphore
# wait per CTRL instruction, but TileContext puts every outstanding proc's
# wait on the single tail Drain.  Emit one-wait nops first instead.
# ---------------------------------------------------------------------------
def _split_drain_and_barrier(self, tick_clock, wait_clock):
    gc = tick_clock.global_clock
    n = len(gc)
    for p in range(n):
        if gc[p] > 0:
            vec = [0] * n
            vec[p] = gc[p]
            nop = self.nc.sync.nop()
            wait_clock.add_sem_waits(nop.ins, ScopedClock({None: VectorClock(vec)}))
    self.nc.sync.drain()
    self.nc.all_engine_barrier()
    assert self.sems is not None
    popped = self.nc._tile_sem_poison_stack.pop()
    assert popped is self._sem_poison
    self.nc.clear_and_free_semaphores(list(self.sems.allocated().values()))
    self.nc.all_engine_barrier()


tile.TileContext._drain_and_barrier = _split_drain_and_barrier


def _legalize_waits(bir: bytes) -> bytes:
    """This walrus build accepts at most ONE semaphore wait per instruction.
    Split every multi-wait instruction by injecting same-engine NoOps (each
    carrying one wait) immediately before it: engines consume their block
    instructions in order, so the nop's wait completes before the real op."""
    import json as _json

    d = _json.loads(bir)
    ctr = 0
    for f in d["functions"]:
        for b in f["blocks"]:
            newl = []
            for inst in b["instructions"]:
                si = inst.get("sync_info")
                w = (si or {}).get("on_wait") or []
                if len(w) > 1:
                    for extra in w[:-1]:
                        ctr += 1
                        newl.append(
                            {
                                "debug": inst.get("debug", 0),
                                "engine": inst["engine"],
                                "ins": [],
                                "outs": [],
                                "name": f"I-wfix{ctr}",
                                "opcode": "NoOp",
                                "sync_info": {"on_update": [], "on_wait": [extra]},
                            }
                        )
                    si["on_wait"] = [w[-1]]
                newl.append(inst)
            b["instructions"] = newl
    return _json.dumps(d).encode()


def _install_wait_legalizer(nc):
    orig = nc.to_json_bytes

    def to_json_bytes_fixed():
        return _legalize_waits(orig())

    nc.to_json_bytes = to_json_bytes_fixed
    return nc


# ---------------------------------------------------------------------------
# Poly fast path
# ---------------------------------------------------------------------------
PCHUNK = 512                        # poly matmul free dim (= one PSUM bank)
PCHUNKS = 49                        # ceil(25000/512)
NPC_PAD = PCHUNKS * PCHUNK          # 25088 (88 zero-pad points per core)
PDEG = 3                            # monomials kept: the deg-4 part of
                                    # L*(1+y+y^2+y^3) adds < 1.2 px worst-case
# canonical monomial order: all (e1,e2,e3) with e1+e2+e3 <= 3 -> 20 rows
PMONOS = sorted(
    (e1, e2, e3)
    for e1 in range(PDEG + 1)
    for e2 in range(PDEG + 1)
    for e3 in range(PDEG + 1)
    if e1 + e2 + e3 <= PDEG
)
PK = len(PMONOS)                    # 20


def _poly_weights(R, t, f, su, sv):
    """[PK, V] f64 coefficient matrices W_u, W_v of the degree-4 series.

    q_u[v,n] = sum_m W_u[m,v] * mu_m(p_n), where
    q_u = (f/(su*a)) * (r0.p + tx) * (1 + y + y^2 + y^3),  y = (r2.p)/a.
    """
    V = R.shape[0]
    r0, r1, r2 = R[:, 0, :], R[:, 1, :], R[:, 2, :]
    tx, ty, tz = t[:, 0], t[:, 1], t[:, 2]
    a = -tz

    def pmul(A, B):
        out = {}
        for ea, ca in A.items():
            for eb, cb in B.items():
                k = (ea[0] + eb[0], ea[1] + eb[1], ea[2] + eb[2])
                out[k] = out.get(k, 0) + ca * cb
        return out

    def padd(A, B):
        out = dict(A)
        for k, c in B.items():
            out[k] = out.get(k, 0) + c
        return out

    g = r2 / a[:, None]
    Y = {(1, 0, 0): g[:, 0], (0, 1, 0): g[:, 1], (0, 0, 1): g[:, 2]}
    Y2 = pmul(Y, Y)
    Y3 = pmul(Y2, Y)
    S = padd(padd({(0, 0, 0): np.ones(V)}, Y), padd(Y2, Y3))
    Lu = {(0, 0, 0): tx, (1, 0, 0): r0[:, 0], (0, 1, 0): r0[:, 1], (0, 0, 1): r0[:, 2]}
    Lv = {(0, 0, 0): ty, (1, 0, 0): r1[:, 0], (0, 1, 0): r1[:, 1], (0, 0, 1): r1[:, 2]}
    au = f / (su * a)
    av = -f / (sv * a)
    Pu = {k: au * c for k, c in pmul(Lu, S).items()}
    Pv = {k: av * c for k, c in pmul(Lv, S).items()}
    Wu = np.stack([Pu.get(m, np.zeros(V)) for m in PMONOS], 0)
    Wv = np.stack([Pv.get(m, np.zeros(V)) for m in PMONOS], 0)
    return Wu, Wv


def _poly_cast_schedule():
    """Per-chunk cast engine: greedy least-loaded with measured per-op costs
    (ACT (1024+202)/1.2 ns, DVE tensor_copy (1024+135)/0.96 ns)."""
    tA = tD = 0.0
    cA, cD = 1022.0, 1161.0
    eng = []
    for _ in range(PCHUNKS):
        if tA + cA <= tD + cD:
            eng.append("A")
            tA += cA
        else:
            eng.append("D")
            tD += cD
    # end on the faster engine: the final cast gates the tail (store +
    # receipt + drain), so swap the last D with the nearest preceding A
    if eng[-1] == "D":
        i = len(eng) - 1 - eng[::-1].index("A")
        eng[i], eng[-1] = "D", "A"
    return eng


def _build_module_poly():
    WCOL = N_VIEWS                  # weight columns prepended to each blob
    BLOBW = WCOL + NPC_PAD
    nc = bass.Bass()
    # blobs: [W (20x128) | monomials (20x25088)] -- embedding the weights
    # in the same tensor lets ONE first-piece DMA deliver both the lhsT and
    # chunk 0, removing a separate weight-DMA from the ramp critical path
    bu_d = nc.declare_dram_parameter("blob_u", [PK, BLOBW], BF16, isOutput=False)
    bv_d = nc.declare_dram_parameter("blob_v", [PK, BLOBW], BF16, isOutput=False)
    out_d = nc.declare_dram_parameter("out", [N_VIEWS, 2 * NPC_PAD], I8, isOutput=True)

    with tile.TileContext(nc) as tc, ExitStack() as ctx:
        const_pool = ctx.enter_context(tc.tile_pool(name="const", bufs=1))
        # one [128, 1024] f32 tile = 2 PSUM banks; 4 bufs = all 8 banks.
        # Small tiles keep matmul latency off the cast critical path.
        psum_pool = ctx.enter_context(tc.tile_pool(name="ps", bufs=4, space="PSUM"))
        out_pool = ctx.enter_context(tc.tile_pool(name="out", bufs=14))

        # blob tiles: u copy at partitions 0-19, v copy at 64-83
        big = const_pool.tile([64 + PK, BLOBW], BF16, tag="mu")

        # warm the ACT spline tables before any input lands
        warm = const_pool.tile([1, 2], F32, tag="warm")
        nc.vector.memset(warm[:], 1.0)
        nc.scalar.copy(warm[0:1, 1:2], warm[0:1, 0:1])

        # PE pre-warm: the HAM clock gate holds the PE at 1.2 GHz until
        # ~3.4 us of sustained activity.  Start that window early with 4
        # dummy matmuls on the UNUSED row strip 1 (memset inputs, no DMA
        # dependency) so the real matmuls hit 2.4 GHz ~2 us sooner.  Only
        # 4 (~1.7 us cold) so the real chunk-0 matmuls never queue behind
        # them on the PE FIFO.
        wsrc = const_pool.tile([32 + PK, PCHUNK], BF16, tag="wsrc")
        nc.vector.memset(wsrc[32 : 32 + PK, :], 0.0)
        wps = psum_pool.tile([N_VIEWS, 2 * PCHUNK], F32, tag="puv")
        for _ in range(4):
            nc.tensor.matmul(
                wps[:, 0:PCHUNK],
                wsrc[32 : 32 + PK, 0:N_VIEWS],
                wsrc[32 : 32 + PK, 0:PCHUNK],
                tile_position=(32, 0),
            )

        # Input loads must NOT ride the scalar ring: the ACT engine FIFO
        # stalls behind each skinny-partition DMA trigger (~1.3 us apiece)
        # and the casts starve.  u blob on the sync HWDGE ring, v blob on
        # the gpsimd SWDGE ring; ACT issues nothing.  Geometric piece
        # sizes: the tiny first piece (weights + chunk 0) gates the first
        # matmul on ~26 KB instead of ~100 KB.
        edges = [0, WCOL + 512, WCOL + 1536, WCOL + 3072, WCOL + 5120,
                 WCOL + 7680, WCOL + 10752, WCOL + 14336, WCOL + 17920,
                 WCOL + 21504, BLOBW]
        for i in range(len(edges) - 1):
            lo, hi = edges[i], edges[i + 1]
            nc.sync.dma_start(big[0:PK, lo:hi], bu_d[:, lo:hi])
            nc.gpsimd.dma_start(big[64 : 64 + PK, lo:hi], bv_d[:, lo:hi])

        lhs_u = big[0:PK, 0:WCOL]
        lhs_v = big[64 : 64 + PK, 0:WCOL]
        mu_u = big[0:PK, WCOL:]
        mu_v = big[64 : 64 + PK, WCOL:]
        eng = _poly_cast_schedule()

        otile = None
        for c in range(PCHUNKS):
            csl = slice(c * PCHUNK, (c + 1) * PCHUNK)
            puv = psum_pool.tile([N_VIEWS, 2 * PCHUNK], F32, tag="puv")
            nc.tensor.matmul(
                puv[:, 0:PCHUNK], lhs_u, mu_u[:, csl], tile_position=(0, 0)
            )
            nc.tensor.matmul(
                puv[:, PCHUNK : 2 * PCHUNK], lhs_v, mu_v[:, csl],
                tile_position=(64, 0),
            )
            half = c % 2
            if half == 0:
                otile = out_pool.tile([N_VIEWS, 4 * PCHUNK], I8, tag="o")
            dst = otile[:, half * 2 * PCHUNK : (half + 1) * 2 * PCHUNK]
            if eng[c] == "A":
                nc.scalar.copy(dst, puv[:])
            else:
                nc.vector.tensor_copy(dst, puv[:])
            # stores split across the sync and gpsimd rings (one ring can't
            # carry loads + all 25 stores), but the LAST gpsimd store must
            # land well before the end: the SWDGE end-of-kernel drain waits
            # on its own last receipt (~3.5 us when a store issues at ~40 us)
            if c >= PCHUNKS - 5:
                # tail: per-chunk stores, sync ring only
                nc.sync.dma_start(
                    out_d[:, c * 2 * PCHUNK : (c + 1) * 2 * PCHUNK], dst
                )
            elif half == 1:
                W = (half + 1) * 2 * PCHUNK
                q = nc.sync if (c // 2) % 2 == 0 else nc.gpsimd
                q.dma_start(
                    out_d[:, (c - half) * 2 * PCHUNK : (c - half) * 2 * PCHUNK + W],
                    otile[:, 0:W],
                )

    return _install_wait_legalizer(nc)


# ---------------------------------------------------------------------------
# Host-side math
# ---------------------------------------------------------------------------
def _euler_to_matrix(e):
    """[V,3] -> [V,3,3], Rx @ Ry @ Rz (same convention as the reference)."""
    x, y, z = e[:, 0], e[:, 1], e[:, 2]
    c1, s1 = np.cos(x), np.sin(x)
    c2, s2 = np.cos(y), np.sin(y)
    c3, s3 = np.cos(z), np.sin(z)
    zero = np.zeros_like(x)
    one = np.ones_like(x)
    Rx = np.stack([one, zero, zero, zero, c1, -s1, zero, s1, c1], -1).reshape(-1, 3, 3)
    Ry = np.stack([c2, zero, s2, zero, one, zero, -s2, zero, c2], -1).reshape(-1, 3, 3)
    Rz = np.stack([c3, -s3, zero, s3, c3, zero, zero, zero, one], -1).reshape(-1, 3, 3)
    return Rx @ Ry @ Rz


def _pack(w, b):
    # -> [KROWS, V] bf16 lhsT: cols per view = [w_hi(3), w_hi(3), w_lo(3),
    # b_hi, b_lo] matching point rows [p_hi(3), p_lo(3), p_hi(3), 1, 1]
    import ml_dtypes

    w_hi = w.astype(ml_dtypes.bfloat16)
    w_lo = (w - w_hi.astype(np.float64)).astype(ml_dtypes.bfloat16)
    b_hi = b.astype(ml_dtypes.bfloat16)
    b_lo = (b - b_hi.astype(np.float64)).astype(ml_dtypes.bfloat16)
    return np.concatenate(
        [w_hi.T, w_hi.T, w_lo.T, b_hi[None, :], b_lo[None, :]], axis=0
    )


def _fold_weights(euler_angles, translations, focal_length, variant, su=None, sv=None):
    """Build the three [KROWS, V] stationary matrices."""
    R = _euler_to_matrix(euler_angles.astype(np.float64))
    t = translations.astype(np.float64)
    f = float(focal_length[0])
    r0, r1, r2 = R[:, 0, :], R[:, 1, :], R[:, 2, :]
    tx, ty, tz = t[:, 0], t[:, 1], t[:, 2]

    if variant == "i8":
        # q_u = (f/s_u)*Xc/znega : the CX fold cancels exactly (CX=512)
        wU = (f / su)[:, None] * r0
        bU = (f / su) * tx
        wV = (-f / sv)[:, None] * r1
        bV = (-f / sv) * ty
    elif variant == "clamp":
        # numerators without the CX/CY fold (added on DVE after the division)
        wU = f * r0
        bU = f * tx
        wV = -f * r1
        bV = -f * ty
    else:  # f16
        wU = f * r0 - CX * r2
        bU = f * tx - CX * tz
        wV = -f * r1 - CY * r2
        bV = -f * ty - CY * tz
    wZ = -r2
    bZ = -tz
    return _pack(wU, bU), _pack(wV, bV), _pack(wZ, bZ)


# ---------------------------------------------------------------------------
# Bass module
# ---------------------------------------------------------------------------
def _build_module(variant):
    i8 = variant == "i8"
    clamp = variant == "clamp"
    ODT = I8 if i8 else F16
    off_pairs = OFF_PAIRS if not clamp else ()

    nc = bass.Bass()
    blob_u = nc.declare_dram_parameter("blob_u", [KROWS, BLOB], BF16, isOutput=False)
    blob_v = nc.declare_dram_parameter("blob_v", [KROWS, BLOB], BF16, isOutput=False)
    blob_z = nc.declare_dram_parameter("blob_z", [KROWS, BLOB], BF16, isOutput=False)
    out = nc.declare_dram_parameter("out", [N_VIEWS, 2 * NPC], ODT, isOutput=True)
    if off_pairs:
        out_off = nc.declare_dram_parameter(
            "out_off", [N_VIEWS, len(off_pairs) * 2 * CHUNK], F16, isOutput=True
        )

    with tile.TileContext(nc) as tc, ExitStack() as ctx:
        const_pool = ctx.enter_context(tc.tile_pool(name="const", bufs=1))
        # PSUM: U,V ring gets 3 bufs (6 banks) so the ACT staging copy of an
        # offloaded chunk is never inside the ring's reuse window; z tiles
        # are per-chunk single banks (2 bufs = 2 banks).  6 + 2 = 8 banks.
        psum_uv = ctx.enter_context(tc.tile_pool(name="psuv", bufs=3, space="PSUM"))
        psum_z = ctx.enter_context(tc.tile_pool(name="psz", bufs=2, space="PSUM"))
        # per-chunk reciprocals in a deep ring: the gpsimd consumer finishes
        # ~5us after its pair starts and must not block recip reuse
        recip_pool = ctx.enter_context(tc.tile_pool(name="recip", bufs=12))
        sb_pool = ctx.enter_context(tc.tile_pool(name="sb", bufs=6))
        # deep output ring: a pair's compute must never wait on the
        # data-landed semaphore of a recent store
        out_pool = ctx.enter_context(tc.tile_pool(name="out", bufs=12))
        # separate ring for the gpsimd-computed chunks: their later finish
        # must not block the sync store queue or ring A
        off_pool = ctx.enter_context(tc.tile_pool(name="off", bufs=6))

        # blob sections land at partition rows 0-10 (u), 32-42 (v), 64-74 (z)
        # so each PE row group streams its own rhs copy.
        btile = const_pool.tile([64 + KROWS, BLOB], BF16, tag="blob")
        # z first: the recip chain (z matmul -> ACT -> DVE) gates chunk 0
        sections = ((64, blob_z), (0, blob_u), (32, blob_v))

        # Tiny weights+first-pair pieces on separate queues so their issue
        # and completion don't serialize; then 12 pieces up-front on the
        # gpsimd (SWDGE) queue (kept under the SWDGE ring depth so no
        # mid-stream drain blocks late pieces), interleaved across sections
        # in consumption order.
        SPLIT = WCOLS + 2 * CHUNK
        for (base, blob), eng in zip(sections, (nc.sync, nc.scalar, nc.sync)):
            eng.dma_start(btile[base : base + KROWS, 0:SPLIT], blob[:, 0:SPLIT])
        GRPS = (6, 10, 16, 16)
        edges = [2]
        for g in GRPS:
            edges.append(edges[-1] + g)
        # u-section pieces ride the scalar HWDGE queue (qACT ring, otherwise
        # unused): input lands via two independent DGE paths (~2x faster than
        # one SWDGE queue).  Only 2 u pieces so the issue slots ahead of the
        # first Reciprocal on the ACT queue stay short.
        for lo_c, hi_c in ((2, 18), (18, 50)):
            nc.scalar.dma_start(
                btile[0:KROWS, WCOLS + lo_c * CHUNK : WCOLS + hi_c * CHUNK],
                blob_u[:, WCOLS + lo_c * CHUNK : WCOLS + hi_c * CHUNK],
            )
        for gi in range(len(GRPS)):
            for base, blob in sections:
                if base == 0:
                    continue
                lo = WCOLS + edges[gi] * CHUNK
                hi = WCOLS + edges[gi + 1] * CHUNK
                nc.gpsimd.dma_start(
                    btile[base : base + KROWS, lo:hi], blob[:, lo:hi]
                )

        ACT_FN = mybir.ActivationFunctionType

        def act_direct(out_ap, in_ap, func, bias=0.0, scale=1.0, alpha=0.0):
            # same lowering as nc.scalar.activation but without the
            # Reciprocal accuracy guard (measured 1.2e-5 rel err on our
            # [1.1, 3.6] domain, far inside the output tolerance)
            eng = nc.scalar
            ins = [eng.lower_ap(in_ap)]
            for val in (bias, scale, alpha):
                ins.append(mybir.ImmediateValue(dtype=mybir.dt.float32, value=val))
            return eng.add_instruction(
                mybir.InstActivation(
                    name=nc.get_next_instruction_name(),
                    func=func,
                    ins=ins,
                    outs=[eng.lower_ap(out_ap)],
                )
            )

        # pre-warm the ACT spline tables (~2.7 us) under the input transfer
        warm = sb_pool.tile([1, 2], F32, tag="warm")
        nc.vector.memset(warm[:], 1.0)
        act_direct(warm[0:1, 1:2], warm[0:1, 0:1], ACT_FN.Reciprocal)

        ZPW = 2 * CHUNK + (BANK - CHUNK)   # 1012
        lhs_u = btile[0:KROWS, 0:WCOLS]
        lhs_v = btile[32 : 32 + KROWS, 0:WCOLS]
        lhs_z = btile[64 : 64 + KROWS, 0:WCOLS]

        def make_recip(c):
            # z matmul + per-chunk Reciprocal into the deep recip ring
            pz = psum_z.tile([N_VIEWS, BANK], F32, tag="pz")
            rhs_z = btile[
                64 : 64 + KROWS, WCOLS + c * CHUNK : WCOLS + (c + 1) * CHUNK
            ]
            nc.tensor.matmul(pz[:, 0:CHUNK], lhs_z, rhs_z, tile_position=(64, 0))
            rtile = recip_pool.tile([N_VIEWS, BANK], BF16, tag="r")
            if clamp:
                zcl = sb_pool.tile([N_VIEWS, BANK], F32, tag="zcl")
                nc.vector.tensor_scalar_max(zcl[:, 0:CHUNK], pz[:, 0:CHUNK], -Z_MAX)
                act_direct(rtile[:, 0:CHUNK], zcl[:, 0:CHUNK], ACT_FN.Reciprocal)
            else:
                act_direct(rtile[:, 0:CHUNK], pz[:, 0:CHUNK], ACT_FN.Reciprocal)
            return rtile

        for p in range(PAIRS):
            c0 = 2 * p
            rtiles = [make_recip(c0), make_recip(c0 + 1)]

            # output tile: planar per pair [u(2p) u(2p+1) | v(2p) v(2p+1)]
            otile = out_pool.tile([N_VIEWS, 4 * CHUNK], ODT, tag="o")
            o3 = otile[:].rearrange("p (two n) -> p two n", two=2)

            for e in range(2):
                c = c0 + e
                puv = psum_uv.tile([N_VIEWS, 2 * BANK], F32, tag="puv")
                rhs = btile[0:KROWS, WCOLS + c * CHUNK : WCOLS + (c + 1) * CHUNK]
                rhs_v = btile[
                    32 : 32 + KROWS, WCOLS + c * CHUNK : WCOLS + (c + 1) * CHUNK
                ]
                nc.tensor.matmul(
                    puv[:, 0:CHUNK], lhs_u, rhs, tile_position=(0, 0)
                )
                nc.tensor.matmul(
                    puv[:, BANK : BANK + CHUNK], lhs_v, rhs_v,
                    tile_position=(32, 0),
                )

                rb = (
                    rtiles[e][:, 0:CHUNK]
                    .unsqueeze(1)
                    .broadcast_to([N_VIEWS, 2, CHUNK])
                )
                odst = o3[:, :, e * CHUNK : (e + 1) * CHUNK]
                if e == 1 and p in OFF_IDX and not clamp:
                    # stage U,V to SBUF (frees the PSUM slot early) and let
                    # the Q7 cores do this chunk's quotients into their own
                    # output ring, stored fp16 from the gpsimd queue itself
                    uvc = sb_pool.tile([N_VIEWS, 2 * BANK], BF16, tag="uvc")
                    act_direct(uvc[:, 0:ZPW], puv[:, 0:ZPW], ACT_FN.Copy)
                    cuv = uvc[:].rearrange(
                        "p (two n) -> p two n", two=2
                    )[:, :, 0:CHUNK]
                    offt = off_pool.tile([N_VIEWS, 2 * CHUNK], F16, tag="g")
                    off3 = offt[:].rearrange("p (two n) -> p two n", two=2)
                    nc.gpsimd.tensor_tensor(off3, cuv, rb, mybir.AluOpType.mult)
                    k = OFF_IDX[p]
                    nc.gpsimd.dma_start(
                        out_off[:, k * 2 * CHUNK : (k + 1) * 2 * CHUNK], offt[:]
                    )
                    continue
                iuv = puv[:].rearrange("p (two n) -> p two n", two=2)[:, :, 0:CHUNK]
                if clamp:
                    tuv = sb_pool.tile([N_VIEWS, 2 * CHUNK], F32, tag="tuv")
                    t3 = tuv[:].rearrange("p (two n) -> p two n", two=2)
                    nc.vector.tensor_tensor(t3, iuv, rb, mybir.AluOpType.mult)
                    nc.vector.tensor_scalar_add(
                        odst[:, 0:1, :], t3[:, 0:1, :], CX
                    )
                    nc.vector.tensor_scalar_add(
                        odst[:, 1:2, :], t3[:, 1:2, :], CY
                    )
                else:
                    nc.vector.tensor_tensor(odst, iuv, rb, mybir.AluOpType.mult)

            # store the pair immediately -- keeps the DMA queues fed;
            # offloaded pairs store only the DVE-computed e=0 half here (the
            # gpsimd half went out fp16 above); the final pairs store per
            # chunk so the tail drains faster
            od = out[:, p * 4 * CHUNK : (p + 1) * 4 * CHUNK]
            od3 = od.rearrange("p (two n) -> p two n", two=2)
            if p in OFF_IDX and not clamp:
                nc.sync.dma_start(od3[:, :, 0:CHUNK], o3[:, :, 0:CHUNK])
            elif p >= PAIRS - 2:
                for e in range(2):
                    nc.sync.dma_start(
                        od3[:, :, e * CHUNK : (e + 1) * CHUNK],
                        o3[:, :, e * CHUNK : (e + 1) * CHUNK],
                    )
            else:
                nc.sync.dma_start(od, otile[:])

    return _install_wait_legalizer(nc)


_module_cache = {}


def _get_module(variant):
    if variant not in _module_cache:
        if variant == "poly":
            _module_cache[variant] = _build_module_poly()
        else:
            _module_cache[variant] = _build_module(variant)
    return _module_cache[variant]


def _kernel_poly(points3d, Rq, t64, f, su, sv, _trace=False):
    import ml_dtypes

    Wu, Wv = _poly_weights(Rq, t64, f, su, sv)       # [PK, V] f64
    Wub = np.ascontiguousarray(Wu.astype(ml_dtypes.bfloat16))
    Wvb = np.ascontiguousarray(Wv.astype(ml_dtypes.bfloat16))

    # monomial matrix from bf16-rounded points (f32 products are near-exact)
    pb = points3d.astype(ml_dtypes.bfloat16).astype(np.float32)  # [N,3]
    pows = []                                        # pows[j][k] = p_j^k
    for j in range(3):
        col = [np.ones(pb.shape[0], np.float32)]
        for _ in range(PDEG):
            col.append(col[-1] * pb[:, j])
        pows.append(col)
    mu = np.empty((PK, pb.shape[0]), np.float32)
    for m, (e1, e2, e3) in enumerate(PMONOS):
        mu[m] = pows[0][e1] * pows[1][e2] * pows[2][e3]
    mu_bf = mu.astype(ml_dtypes.bfloat16)            # [PK, N]

    nc = _get_module("poly")
    in_maps = []
    for c in range(N_CORES):
        mu_core = np.zeros((PK, NPC_PAD), dtype=ml_dtypes.bfloat16)
        mu_core[:, :NPC] = mu_bf[:, c * NPC : (c + 1) * NPC]
        in_maps.append(
            {
                "blob_u": np.ascontiguousarray(
                    np.concatenate([Wub, mu_core], axis=1)
                ),
                "blob_v": np.ascontiguousarray(
                    np.concatenate([Wvb, mu_core], axis=1)
                ),
            }
        )

    res = run_bass_kernel_spmd(
        nc, in_maps, core_ids=list(range(N_CORES)), trace=_trace
    )

    su32 = su.astype(np.float32)[:, None]
    sv32 = sv.astype(np.float32)[:, None]
    full = np.empty((N_VIEWS, N_POINTS, 2), dtype=np.float32)
    for c in range(N_CORES):
        q = res.results[c]["out"].reshape(N_VIEWS, PCHUNKS, 2, PCHUNK)
        qu = q[:, :, 0, :].reshape(N_VIEWS, NPC_PAD)[:, :NPC].astype(np.float32)
        qv = q[:, :, 1, :].reshape(N_VIEWS, NPC_PAD)[:, :NPC].astype(np.float32)
        full[:, c * NPC : (c + 1) * NPC, 0] = CX + su32 * qu
        full[:, c * NPC : (c + 1) * NPC, 1] = CY + sv32 * qv
    if _trace:
        return full, res
    return full


# ---------------------------------------------------------------------------
# Entry point
# ---------------------------------------------------------------------------
def kernel(points3d, euler_angles, translations, focal_length, _trace=False):
    points3d = np.asarray(points3d, dtype=np.float32)
    euler_angles = np.asarray(euler_angles, dtype=np.float32)
    translations = np.asarray(translations, dtype=np.float32)
    focal_length = np.asarray(focal_length, dtype=np.float32)
    f = float(focal_length[0])

    # Is the Z clamp provably inactive?  znega = -(r2.p + tz) >= min_v(-tz -
    # |r2|*max|p|).  The fast paths fold CX/CY into the matmul, which is only
    # exact when no point clamps.
    Rq = _euler_to_matrix(euler_angles.astype(np.float64))
    tz = translations[:, 2].astype(np.float64)
    tx = translations[:, 0].astype(np.float64)
    ty = translations[:, 1].astype(np.float64)
    r2n = np.linalg.norm(Rq[:, 2, :], axis=1)
    pmax = float(np.linalg.norm(points3d.astype(np.float64), axis=1).max())
    znega_lo_v = -tz - r2n * pmax
    znega_lo = float(znega_lo_v.min())
    if znega_lo < max(-Z_MAX * 10.0, 1e-3):
        variant = "clamp"
    else:
        # per-view int8 scales from rigorous bounds (|row(R)| = 1)
        bu = f * (pmax + np.abs(tx)) / znega_lo_v
        bv = f * (pmax + np.abs(ty)) / znega_lo_v
        su = bu * 1.03 / 127.0
        sv = bv * 1.03 / 127.0
        # quantization step too coarse -> prefer the fp16 variant
        variant = "i8" if max(su.max(), sv.max()) < 4.0 else "f16"

    # poly fast path: series residual y^4/(1-y) must be small and the
    # int8 scale fine enough
    if variant == "i8":
        a_v = -tz
        ymax = float((r2n * pmax / a_v).max())
        if a_v.min() > 0.5 and ymax <= 0.33:
            t64 = translations.astype(np.float64)
            return _kernel_poly(
                points3d.astype(np.float64), Rq, t64, f, su, sv, _trace
            )

    if variant == "i8":
        Wu, Wv, Wz = _fold_weights(
            euler_angles, translations, focal_length, "i8", su, sv
        )
    else:
        Wu, Wv, Wz = _fold_weights(euler_angles, translations, focal_length, variant)

    import ml_dtypes

    pT = points3d.T                                   # [3, N] f32
    p_hi = pT.astype(ml_dtypes.bfloat16)              # [3, N]
    p_lo = (pT - p_hi.astype(np.float32)).astype(ml_dtypes.bfloat16)
    ones = np.ones((1, N_POINTS), dtype=ml_dtypes.bfloat16)
    pk = np.concatenate([p_hi, p_lo, p_hi, ones, ones], axis=0)  # [KROWS, N]

    nc = _get_module(variant)
    in_maps = []
    for c in range(N_CORES):
        sl = pk[:, c * NPC : (c + 1) * NPC]
        in_maps.append(
            {
                "blob_u": np.ascontiguousarray(np.concatenate([Wu, sl], axis=1)),
                "blob_v": np.ascontiguousarray(np.concatenate([Wv, sl], axis=1)),
                "blob_z": np.ascontiguousarray(np.concatenate([Wz, sl], axis=1)),
            }
        )

    res = run_bass_kernel_spmd(
        nc, in_maps, core_ids=list(range(N_CORES)), trace=_trace
    )

    full = np.empty((N_VIEWS, N_POINTS, 2), dtype=np.float32)
    for c in range(N_CORES):
        # DRAM layout: 25 pairs x [u(2p) u(2p+1) | v(2p) v(2p+1)]
        q = res.results[c]["out"].reshape(N_VIEWS, PAIRS, 2, 2, CHUNK)
        blk = (
            q.transpose(0, 1, 3, 4, 2)                 # [V, pair, e, n, uv]
            .reshape(N_VIEWS, NPC, 2)
            .astype(np.float32)
        )
        if variant == "i8":
            qo = res.results[c]["out_off"].astype(np.float32)
            qo = qo.reshape(N_VIEWS, len(OFF_PAIRS), 2, CHUNK)
            for k, p in enumerate(OFF_PAIRS):
                cidx = 2 * p + 1
                blk[:, cidx * CHUNK : (cidx + 1) * CHUNK, 0] = qo[:, k, 0, :]
                blk[:, cidx * CHUNK : (cidx + 1) * CHUNK, 1] = qo[:, k, 1, :]
            blk[:, :, 0] = CX + su[:, None].astype(np.float32) * blk[:, :, 0]
            blk[:, :, 1] = CY + sv[:, None].astype(np.float32) * blk[:, :, 1]
        elif variant == "f16":
            qo = res.results[c]["out_off"].astype(np.float32)
            qo = qo.reshape(N_VIEWS, len(OFF_PAIRS), 2, CHUNK)
            for k, p in enumerate(OFF_PAIRS):
                cidx = 2 * p + 1
                blk[:, cidx * CHUNK : (cidx + 1) * CHUNK, 0] = qo[:, k, 0, :]
                blk[:, cidx * CHUNK : (cidx + 1) * CHUNK, 1] = qo[:, k, 1, :]
        full[:, c * NPC : (c + 1) * NPC, :] = blk
    if _trace:
        return full, res
    return full



# revision 35
# speedup vs baseline: 1.0232x; 1.0232x over previous
"""Bundle-adjustment forward projection on 8 Trainium2 NeuronCores.

reference:  R = euler_to_matrix(euler_angles)            [V,3,3]
            pc = einsum('nj,vij->vni', points3d, R) + t  [V,N,3]
            Zc = min(pc_z, -1e-4)
            u = -f*Xc/Zc + CX ; v = f*Yc/Zc + CY         -> [V,N,2]

FAST PATH ("poly"): the projective division is absorbed into the matmul.
With a = -tz > 0 and y = (r2.p)/a bounded (|y| <= ~0.25 for this
problem's geometry), 1/znega = (1/a)(1 + y + y^2 + y^3) + O(y^4), so the
quantized output q_u = (u - CX)/s_u = (f/(s_u a))(r0.p + tx)(1+y+y^2+y^3)
is a polynomial in the point coordinates, truncated to the K=20 monomials
of degree <= 3 (the dropped deg-4 part is < 1.2 px worst-case):
q_u[v,n] = sum_m W_u[m,v] * mu_m(p_n).  The device then does NO
reciprocals and NO elementwise multiplies -- just two K=20 bf16 matmul
streams (u on PE row strip 0 via tile_position (0,0), v on strip 2 via
(64,0), overlapping) plus exactly one PSUM f32 -> SBUF int8 cast per
output element.  Total error ~2.3 px on a ~778 px scale (rel 3.0e-3)
vs the 2e-2 gate.

Poly device pipeline per 512-point chunk: 2 matmuls (u, v) into a
[128,1024] 2-bank PSUM tile (bufs=4 = all 8 banks -- small tiles keep
matmul latency OFF the cast critical path), one FD=1024 cast
(ScalarE activation-Copy or VectorE tensor_copy, both 1x rate on PSUM
f32 src, split 26/23 by measured speed and running CONCURRENTLY on
different banks), one 256 KB int8 store per chunk pair.  The cast pair
is the wall: both engines run ~95% dense for ~28 us.  N is padded
25000 -> 49*512 = 25088 per core.

Measured phase budget (of ~44 us): ~7.5 us fixed NEFF/framework
preamble, ~2.5 us first-input DMA round trip (weights ride in the same
blob as chunk 0, descriptor gen + ~2 us completion receipt), ~29 us
cast-bound steady state, ~4 us tail (last store receipt + drains; the
cast schedule ends on the faster ScalarE).  Things that DON'T work:
loads on the scalar ring (ACT's FIFO stalls ~1.3 us per skinny DMA
trigger and the casts starve); all stores on one ring (~0.85 us/store
ring time saturates it); late stores on the gpsimd ring (the SWDGE
end-drain waits ~3.5 us on its own last receipt); N=1024 matmuls
(2-bank PSUM output -- builds in bass but the NEFF compiler rejects
it); splitting the final chunk's cast/store across engines (regressed).

The legacy reciprocal-based kernel below is kept as a fallback for
inputs where the series gates fail (z-clamp possible, |y| too large, or
int8 scale too coarse).  Its design notes follow:

The kernel quantizes the output: u = CX + s_u[view] * q where q is stored
as int8 and s_u is a per-view scale folded into the matmul weights on the
host (CX = img/2 = 512 cancels exactly: q = (f/s_u)*Xc * (1/znega)).
The worst-case quantization error is ~2 px on a ~770 px scale (~5e-3
relative, gate is 2e-2).  This cuts the HBM store to 6.4 MB/core.  If
the host-side bound makes the scale too coarse (or the Z clamp can
fire), it falls back to an fp16 variant.

Sharding: N=200000 points split across 8 cores (25000 each); every core
computes all V=128 views so the SBUF partition dim is fully used.

PE: u, v, z matmuls live on three separate 32-row groups of the PE array
(tile_position rows 0/32/64) so the three 500-column streams run
concurrently.  Each row group has its own copy of the point columns in
SBUF partitions 0-10 / 32-42 / 64-74.

Pipeline per 500-point chunk: z matmul -> per-chunk ACT Reciprocal into
a deep ring -> u,v matmuls into a 3-buf PSUM ring -> one DVE broadcast
tensor_tensor computing both quotients (fp32 PSUM x recip -> int8 SBUF)
-> per-pair 256 KB store.  Chunk e=1 of every other mid-stream pair is
computed by the GPSIMD engine instead (ACT stages U,V to SBUF bf16, Q7
multiplies into its own ring and stores fp16 via the SWDGE queue),
taking ~11 us off the DVE critical path.

Hard-won scheduling facts baked in here: (1) deep rings everywhere --
compute must never wait on a store's HBM-receipt semaphore (~3.5 us) or
on the 8 DMAHW completion lanes, so the output ring is 12 pairs deep;
(2) input rides TWO DGE paths in parallel (z+v sections on the gpsimd
SWDGE queue, u section on the scalar HWDGE ring which is otherwise
unused) -- a single queue serializes the 1.66 MB input behind ~93 GB/s
and starves the early pipeline; (3) the offloaded chunks' fp16 halves
store from the gpsimd queue itself so the sync store FIFO never waits
on the late Q7 finish.  Engines measure ~47/48/43/38 us busy
(DVE/ACT/PE/GPSIMD) over a ~66 us kernel -- the DVE stream and the ACT
recip+staging stream are the twin walls.

Numerics: inputs ship as a bf16 hi/lo split (K=11 rows against weight
columns [w_hi, w_hi, w_lo, b_hi, b_lo]); only w_lo*p_lo (~2^-18) is
dropped.

Walrus in this build accepts at most ONE semaphore wait per instruction:
TileContext's tail drain is patched to split its waits into one-wait
nops, and a serialized-BIR rewriter injects same-engine NoOps for any
remaining multi-wait instruction.
"""

import numpy as np
from contextlib import ExitStack

import concourse.bass as bass
import concourse.tile as tile
from concourse import mybir
from concourse.bass_utils import run_bass_kernel_spmd
from concourse.vector_clock import ScopedClock, VectorClock

CX = 512.0
CY = 512.0
Z_MAX = -1e-4

N_CORES = 8
N_POINTS = 200000
N_VIEWS = 128
NPC = N_POINTS // N_CORES          # 25000 points per core
CHUNK = 500                        # matmul free dim (fits one PSUM bank)
CHUNKS = NPC // CHUNK              # 50
PAIRS = CHUNKS // 2                # 25
WCOLS = N_VIEWS                    # weight columns per blob section
BLOB = WCOLS + NPC                 # 25128 cols per 11-row section
KROWS = 11
BANK = 512

# chunk e=1 of these pairs runs on the GPSIMD engine instead of the DVE
OFF_PAIRS = (2, 4, 6, 8, 10, 12, 14, 16, 18, 20)
OFF_IDX = {p: i for i, p in enumerate(OFF_PAIRS)}

F32 = mybir.dt.float32
F16 = mybir.dt.float16
I8 = mybir.dt.int8
BF16 = mybir.dt.bfloat16


# ---------------------------------------------------------------------------
# Tile tail-drain workaround: this walrus build only accepts ONE semaphore
# wait per CTRL instruction, but TileContext puts every outstanding proc's
# wait on the single tail Drain.  Emit one-wait nops first instead.
# ---------------------------------------------------------------------------
def _split_drain_and_barrier(self, tick_clock, wait_clock):
    gc = tick_clock.global_clock
    n = len(gc)
    for p in range(n):
        if gc[p] > 0:
            vec = [0] * n
            vec[p] = gc[p]
            nop = self.nc.sync.nop()
            wait_clock.add_sem_waits(nop.ins, ScopedClock({None: VectorClock(vec)}))
    self.nc.sync.drain()
    self.nc.all_engine_barrier()
    assert self.sems is not None
    popped = self.nc._tile_sem_poison_stack.pop()
    assert popped is self._sem_poison
    self.nc.clear_and_free_semaphores(list(self.sems.allocated().values()))
    self.nc.all_engine_barrier()


tile.TileContext._drain_and_barrier = _split_drain_and_barrier


def _legalize_waits(bir: bytes) -> bytes:
    """This walrus build accepts at most ONE semaphore wait per instruction.
    Split every multi-wait instruction by injecting same-engine NoOps (each
    carrying one wait) immediately before it: engines consume their block
    instructions in order, so the nop's wait completes before the real op."""
    import json as _json

    d = _json.loads(bir)
    ctr = 0
    for f in d["functions"]:
        for b in f["blocks"]:
            newl = []
            for inst in b["instructions"]:
                si = inst.get("sync_info")
                w = (si or {}).get("on_wait") or []
                if len(w) > 1:
                    for extra in w[:-1]:
                        ctr += 1
                        newl.append(
                            {
                                "debug": inst.get("debug", 0),
                                "engine": inst["engine"],
                                "ins": [],
                                "outs": [],
                                "name": f"I-wfix{ctr}",
                                "opcode": "NoOp",
                                "sync_info": {"on_update": [], "on_wait": [extra]},
                            }
                        )
                    si["on_wait"] = [w[-1]]
                newl.append(inst)
            b["instructions"] = newl
    return _json.dumps(d).encode()


def _install_wait_legalizer(nc):
    orig = nc.to_json_bytes

    def to_json_bytes_fixed():
        return _legalize_waits(orig())

    nc.to_json_bytes = to_json_bytes_fixed
    return nc


# ---------------------------------------------------------------------------
# Poly fast path
# ---------------------------------------------------------------------------
PCHUNK = 512                        # poly matmul free dim (= one PSUM bank)
PCHUNKS = 49                        # ceil(25000/512)
NPC_PAD = PCHUNKS * PCHUNK          # 25088 (88 zero-pad points per core)
PDEG = 3                            # monomials kept: the deg-4 part of
                                    # L*(1+y+y^2+y^3) adds < 1.2 px worst-case
# canonical monomial order: all (e1,e2,e3) with e1+e2+e3 <= 3 -> 20 rows
PMONOS = sorted(
    (e1, e2, e3)
    for e1 in range(PDEG + 1)
    for e2 in range(PDEG + 1)
    for e3 in range(PDEG + 1)
    if e1 + e2 + e3 <= PDEG
)
PK = len(PMONOS)                    # 20


def _poly_weights(R, t, f, su, sv):
    """[PK, V] f64 coefficient matrices W_u, W_v of the degree-4 series.

    q_u[v,n] = sum_m W_u[m,v] * mu_m(p_n), where
    q_u = (f/(su*a)) * (r0.p + tx) * (1 + y + y^2 + y^3),  y = (r2.p)/a.
    """
    V = R.shape[0]
    r0, r1, r2 = R[:, 0, :], R[:, 1, :], R[:, 2, :]
    tx, ty, tz = t[:, 0], t[:, 1], t[:, 2]
    a = -tz

    def pmul(A, B):
        out = {}
        for ea, ca in A.items():
            for eb, cb in B.items():
                k = (ea[0] + eb[0], ea[1] + eb[1], ea[2] + eb[2])
                out[k] = out.get(k, 0) + ca * cb
        return out

    def padd(A, B):
        out = dict(A)
        for k, c in B.items():
            out[k] = out.get(k, 0) + c
        return out

    g = r2 / a[:, None]
    Y = {(1, 0, 0): g[:, 0], (0, 1, 0): g[:, 1], (0, 0, 1): g[:, 2]}
    Y2 = pmul(Y, Y)
    Y3 = pmul(Y2, Y)
    S = padd(padd({(0, 0, 0): np.ones(V)}, Y), padd(Y2, Y3))
    Lu = {(0, 0, 0): tx, (1, 0, 0): r0[:, 0], (0, 1, 0): r0[:, 1], (0, 0, 1): r0[:, 2]}
    Lv = {(0, 0, 0): ty, (1, 0, 0): r1[:, 0], (0, 1, 0): r1[:, 1], (0, 0, 1): r1[:, 2]}
    au = f / (su * a)
    av = -f / (sv * a)
    Pu = {k: au * c for k, c in pmul(Lu, S).items()}
    Pv = {k: av * c for k, c in pmul(Lv, S).items()}
    Wu = np.stack([Pu.get(m, np.zeros(V)) for m in PMONOS], 0)
    Wv = np.stack([Pv.get(m, np.zeros(V)) for m in PMONOS], 0)
    return Wu, Wv


def _poly_cast_schedule():
    """Per-chunk cast engine: greedy least-loaded with measured per-op costs
    (ACT (1024+202)/1.2 ns, DVE tensor_copy (1024+135)/0.96 ns)."""
    tA = tD = 0.0
    cA, cD = 1022.0, 1161.0
    eng = []
    for _ in range(PCHUNKS):
        if tA + cA <= tD + cD:
            eng.append("A")
            tA += cA
        else:
            eng.append("D")
            tD += cD
    # end on the faster engine: the final cast gates the tail (store +
    # receipt + drain), so swap the last D with the nearest preceding A
    if eng[-1] == "D":
        i = len(eng) - 1 - eng[::-1].index("A")
        eng[i], eng[-1] = "D", "A"
    return eng


def _build_module_poly():
    WCOL = N_VIEWS                  # weight columns prepended to each blob
    BLOBW = WCOL + NPC_PAD
    nc = bass.Bass()
    # blobs: [W (20x128) | monomials (20x25088)] -- embedding the weights
    # in the same tensor lets ONE first-piece DMA deliver both the lhsT and
    # chunk 0, removing a separate weight-DMA from the ramp critical path
    bu_d = nc.declare_dram_parameter("blob_u", [PK, BLOBW], BF16, isOutput=False)
    bv_d = nc.declare_dram_parameter("blob_v", [PK, BLOBW], BF16, isOutput=False)
    out_d = nc.declare_dram_parameter("out", [N_VIEWS, 2 * NPC_PAD], I8, isOutput=True)

    with tile.TileContext(nc) as tc, ExitStack() as ctx:
        const_pool = ctx.enter_context(tc.tile_pool(name="const", bufs=1))
        # one [128, 1024] f32 tile = 2 PSUM banks; 4 bufs = all 8 banks.
        # Small tiles keep matmul latency off the cast critical path.
        psum_pool = ctx.enter_context(tc.tile_pool(name="ps", bufs=4, space="PSUM"))
        out_pool = ctx.enter_context(tc.tile_pool(name="out", bufs=14))

        # blob tiles: u copy at partitions 0-19, v copy at 64-83
        big = const_pool.tile([64 + PK, BLOBW], BF16, tag="mu")

        # warm the ACT spline tables before any input lands
        warm = const_pool.tile([1, 2], F32, tag="warm")
        nc.vector.memset(warm[:], 1.0)
        nc.scalar.copy(warm[0:1, 1:2], warm[0:1, 0:1])

        # Input loads must NOT ride the scalar ring: the ACT engine FIFO
        # stalls behind each skinny-partition DMA trigger (~1.3 us apiece)
        # and the casts starve.  u blob on the sync HWDGE ring, v blob on
        # the gpsimd SWDGE ring; ACT issues nothing.  Geometric piece
        # sizes: the tiny first piece (weights + chunk 0) gates the first
        # matmul on ~26 KB instead of ~100 KB.
        edges = [0, WCOL + 512, WCOL + 1536, WCOL + 3072, WCOL + 5120,
                 WCOL + 7680, WCOL + 10752, WCOL + 14336, WCOL + 17920,
                 WCOL + 21504, BLOBW]
        for i in range(len(edges) - 1):
            lo, hi = edges[i], edges[i + 1]
            nc.sync.dma_start(big[0:PK, lo:hi], bu_d[:, lo:hi])
            nc.gpsimd.dma_start(big[64 : 64 + PK, lo:hi], bv_d[:, lo:hi])

        lhs_u = big[0:PK, 0:WCOL]
        lhs_v = big[64 : 64 + PK, 0:WCOL]
        mu_u = big[0:PK, WCOL:]
        mu_v = big[64 : 64 + PK, WCOL:]
        eng = _poly_cast_schedule()

        otile = None
        for c in range(PCHUNKS):
            csl = slice(c * PCHUNK, (c + 1) * PCHUNK)
            puv = psum_pool.tile([N_VIEWS, 2 * PCHUNK], F32, tag="puv")
            nc.tensor.matmul(
                puv[:, 0:PCHUNK], lhs_u, mu_u[:, csl], tile_position=(0, 0)
            )
            nc.tensor.matmul(
                puv[:, PCHUNK : 2 * PCHUNK], lhs_v, mu_v[:, csl],
                tile_position=(64, 0),
            )
            half = c % 2
            if half == 0:
                otile = out_pool.tile([N_VIEWS, 4 * PCHUNK], I8, tag="o")
            dst = otile[:, half * 2 * PCHUNK : (half + 1) * 2 * PCHUNK]
            if eng[c] == "A":
                nc.scalar.copy(dst, puv[:])
            else:
                nc.vector.tensor_copy(dst, puv[:])
            # stores split across the sync and gpsimd rings (one ring can't
            # carry loads + all 25 stores), but the LAST gpsimd store must
            # land well before the end: the SWDGE end-of-kernel drain waits
            # on its own last receipt (~3.5 us when a store issues at ~40 us)
            if c >= PCHUNKS - 5:
                # tail: per-chunk stores, sync ring only
                nc.sync.dma_start(
                    out_d[:, c * 2 * PCHUNK : (c + 1) * 2 * PCHUNK], dst
                )
            elif half == 1:
                W = (half + 1) * 2 * PCHUNK
                q = nc.sync if (c // 2) % 2 == 0 else nc.gpsimd
                q.dma_start(
                    out_d[:, (c - half) * 2 * PCHUNK : (c - half) * 2 * PCHUNK + W],
                    otile[:, 0:W],
                )

    return _install_wait_legalizer(nc)


# ---------------------------------------------------------------------------
# Host-side math
# ---------------------------------------------------------------------------
def _euler_to_matrix(e):
    """[V,3] -> [V,3,3], Rx @ Ry @ Rz (same convention as the reference)."""
    x, y, z = e[:, 0], e[:, 1], e[:, 2]
    c1, s1 = np.cos(x), np.sin(x)
    c2, s2 = np.cos(y), np.sin(y)
    c3, s3 = np.cos(z), np.sin(z)
    zero = np.zeros_like(x)
    one = np.ones_like(x)
    Rx = np.stack([one, zero, zero, zero, c1, -s1, zero, s1, c1], -1).reshape(-1, 3, 3)
    Ry = np.stack([c2, zero, s2, zero, one, zero, -s2, zero, c2], -1).reshape(-1, 3, 3)
    Rz = np.stack([c3, -s3, zero, s3, c3, zero, zero, zero, one], -1).reshape(-1, 3, 3)
    return Rx @ Ry @ Rz


def _pack(w, b):
    # -> [KROWS, V] bf16 lhsT: cols per view = [w_hi(3), w_hi(3), w_lo(3),
    # b_hi, b_lo] matching point rows [p_hi(3), p_lo(3), p_hi(3), 1, 1]
    import ml_dtypes

    w_hi = w.astype(ml_dtypes.bfloat16)
    w_lo = (w - w_hi.astype(np.float64)).astype(ml_dtypes.bfloat16)
    b_hi = b.astype(ml_dtypes.bfloat16)
    b_lo = (b - b_hi.astype(np.float64)).astype(ml_dtypes.bfloat16)
    return np.concatenate(
        [w_hi.T, w_hi.T, w_lo.T, b_hi[None, :], b_lo[None, :]], axis=0
    )


def _fold_weights(euler_angles, translations, focal_length, variant, su=None, sv=None):
    """Build the three [KROWS, V] stationary matrices."""
    R = _euler_to_matrix(euler_angles.astype(np.float64))
    t = translations.astype(np.float64)
    f = float(focal_length[0])
    r0, r1, r2 = R[:, 0, :], R[:, 1, :], R[:, 2, :]
    tx, ty, tz = t[:, 0], t[:, 1], t[:, 2]

    if variant == "i8":
        # q_u = (f/s_u)*Xc/znega : the CX fold cancels exactly (CX=512)
        wU = (f / su)[:, None] * r0
        bU = (f / su) * tx
        wV = (-f / sv)[:, None] * r1
        bV = (-f / sv) * ty
    elif variant == "clamp":
        # numerators without the CX/CY fold (added on DVE after the division)
        wU = f * r0
        bU = f * tx
        wV = -f * r1
        bV = -f * ty
    else:  # f16
        wU = f * r0 - CX * r2
        bU = f * tx - CX * tz
        wV = -f * r1 - CY * r2
        bV = -f * ty - CY * tz
    wZ = -r2
    bZ = -tz
    return _pack(wU, bU), _pack(wV, bV), _pack(wZ, bZ)


# ---------------------------------------------------------------------------
# Bass module
# ---------------------------------------------------------------------------
def _build_module(variant):
    i8 = variant == "i8"
    clamp = variant == "clamp"
    ODT = I8 if i8 else F16
    off_pairs = OFF_PAIRS if not clamp else ()

    nc = bass.Bass()
    blob_u = nc.declare_dram_parameter("blob_u", [KROWS, BLOB], BF16, isOutput=False)
    blob_v = nc.declare_dram_parameter("blob_v", [KROWS, BLOB], BF16, isOutput=False)
    blob_z = nc.declare_dram_parameter("blob_z", [KROWS, BLOB], BF16, isOutput=False)
    out = nc.declare_dram_parameter("out", [N_VIEWS, 2 * NPC], ODT, isOutput=True)
    if off_pairs:
        out_off = nc.declare_dram_parameter(
            "out_off", [N_VIEWS, len(off_pairs) * 2 * CHUNK], F16, isOutput=True
        )

    with tile.TileContext(nc) as tc, ExitStack() as ctx:
        const_pool = ctx.enter_context(tc.tile_pool(name="const", bufs=1))
        # PSUM: U,V ring gets 3 bufs (6 banks) so the ACT staging copy of an
        # offloaded chunk is never inside the ring's reuse window; z tiles
        # are per-chunk single banks (2 bufs = 2 banks).  6 + 2 = 8 banks.
        psum_uv = ctx.enter_context(tc.tile_pool(name="psuv", bufs=3, space="PSUM"))
        psum_z = ctx.enter_context(tc.tile_pool(name="psz", bufs=2, space="PSUM"))
        # per-chunk reciprocals in a deep ring: the gpsimd consumer finishes
        # ~5us after its pair starts and must not block recip reuse
        recip_pool = ctx.enter_context(tc.tile_pool(name="recip", bufs=12))
        sb_pool = ctx.enter_context(tc.tile_pool(name="sb", bufs=6))
        # deep output ring: a pair's compute must never wait on the
        # data-landed semaphore of a recent store
        out_pool = ctx.enter_context(tc.tile_pool(name="out", bufs=12))
        # separate ring for the gpsimd-computed chunks: their later finish
        # must not block the sync store queue or ring A
        off_pool = ctx.enter_context(tc.tile_pool(name="off", bufs=6))

        # blob sections land at partition rows 0-10 (u), 32-42 (v), 64-74 (z)
        # so each PE row group streams its own rhs copy.
        btile = const_pool.tile([64 + KROWS, BLOB], BF16, tag="blob")
        # z first: the recip chain (z matmul -> ACT -> DVE) gates chunk 0
        sections = ((64, blob_z), (0, blob_u), (32, blob_v))

        # Tiny weights+first-pair pieces on separate queues so their issue
        # and completion don't serialize; then 12 pieces up-front on the
        # gpsimd (SWDGE) queue (kept under the SWDGE ring depth so no
        # mid-stream drain blocks late pieces), interleaved across sections
        # in consumption order.
        SPLIT = WCOLS + 2 * CHUNK
        for (base, blob), eng in zip(sections, (nc.sync, nc.scalar, nc.sync)):
            eng.dma_start(btile[base : base + KROWS, 0:SPLIT], blob[:, 0:SPLIT])
        GRPS = (6, 10, 16, 16)
        edges = [2]
        for g in GRPS:
            edges.append(edges[-1] + g)
        # u-section pieces ride the scalar HWDGE queue (qACT ring, otherwise
        # unused): input lands via two independent DGE paths (~2x faster than
        # one SWDGE queue).  Only 2 u pieces so the issue slots ahead of the
        # first Reciprocal on the ACT queue stay short.
        for lo_c, hi_c in ((2, 18), (18, 50)):
            nc.scalar.dma_start(
                btile[0:KROWS, WCOLS + lo_c * CHUNK : WCOLS + hi_c * CHUNK],
                blob_u[:, WCOLS + lo_c * CHUNK : WCOLS + hi_c * CHUNK],
            )
        for gi in range(len(GRPS)):
            for base, blob in sections:
                if base == 0:
                    continue
                lo = WCOLS + edges[gi] * CHUNK
                hi = WCOLS + edges[gi + 1] * CHUNK
                nc.gpsimd.dma_start(
                    btile[base : base + KROWS, lo:hi], blob[:, lo:hi]
                )

        ACT_FN = mybir.ActivationFunctionType

        def act_direct(out_ap, in_ap, func, bias=0.0, scale=1.0, alpha=0.0):
            # same lowering as nc.scalar.activation but without the
            # Reciprocal accuracy guard (measured 1.2e-5 rel err on our
            # [1.1, 3.6] domain, far inside the output tolerance)
            eng = nc.scalar
            ins = [eng.lower_ap(in_ap)]
            for val in (bias, scale, alpha):
                ins.append(mybir.ImmediateValue(dtype=mybir.dt.float32, value=val))
            return eng.add_instruction(
                mybir.InstActivation(
                    name=nc.get_next_instruction_name(),
                    func=func,
                    ins=ins,
                    outs=[eng.lower_ap(out_ap)],
                )
            )

        # pre-warm the ACT spline tables (~2.7 us) under the input transfer
        warm = sb_pool.tile([1, 2], F32, tag="warm")
        nc.vector.memset(warm[:], 1.0)
        act_direct(warm[0:1, 1:2], warm[0:1, 0:1], ACT_FN.Reciprocal)

        ZPW = 2 * CHUNK + (BANK - CHUNK)   # 1012
        lhs_u = btile[0:KROWS, 0:WCOLS]
        lhs_v = btile[32 : 32 + KROWS, 0:WCOLS]
        lhs_z = btile[64 : 64 + KROWS, 0:WCOLS]

        def make_recip(c):
            # z matmul + per-chunk Reciprocal into the deep recip ring
            pz = psum_z.tile([N_VIEWS, BANK], F32, tag="pz")
            rhs_z = btile[
                64 : 64 + KROWS, WCOLS + c * CHUNK : WCOLS + (c + 1) * CHUNK
            ]
            nc.tensor.matmul(pz[:, 0:CHUNK], lhs_z, rhs_z, tile_position=(64, 0))
            rtile = recip_pool.tile([N_VIEWS, BANK], BF16, tag="r")
            if clamp:
                zcl = sb_pool.tile([N_VIEWS, BANK], F32, tag="zcl")
                nc.vector.tensor_scalar_max(zcl[:, 0:CHUNK], pz[:, 0:CHUNK], -Z_MAX)
                act_direct(rtile[:, 0:CHUNK], zcl[:, 0:CHUNK], ACT_FN.Reciprocal)
            else:
                act_direct(rtile[:, 0:CHUNK], pz[:, 0:CHUNK], ACT_FN.Reciprocal)
            return rtile

        for p in range(PAIRS):
            c0 = 2 * p
            rtiles = [make_recip(c0), make_recip(c0 + 1)]

            # output tile: planar per pair [u(2p) u(2p+1) | v(2p) v(2p+1)]
            otile = out_pool.tile([N_VIEWS, 4 * CHUNK], ODT, tag="o")
            o3 = otile[:].rearrange("p (two n) -> p two n", two=2)

            for e in range(2):
                c = c0 + e
                puv = psum_uv.tile([N_VIEWS, 2 * BANK], F32, tag="puv")
                rhs = btile[0:KROWS, WCOLS + c * CHUNK : WCOLS + (c + 1) * CHUNK]
                rhs_v = btile[
                    32 : 32 + KROWS, WCOLS + c * CHUNK : WCOLS + (c + 1) * CHUNK
                ]
                nc.tensor.matmul(
                    puv[:, 0:CHUNK], lhs_u, rhs, tile_position=(0, 0)
                )
                nc.tensor.matmul(
                    puv[:, BANK : BANK + CHUNK], lhs_v, rhs_v,
                    tile_position=(32, 0),
                )

                rb = (
                    rtiles[e][:, 0:CHUNK]
                    .unsqueeze(1)
                    .broadcast_to([N_VIEWS, 2, CHUNK])
                )
                odst = o3[:, :, e * CHUNK : (e + 1) * CHUNK]
                if e == 1 and p in OFF_IDX and not clamp:
                    # stage U,V to SBUF (frees the PSUM slot early) and let
                    # the Q7 cores do this chunk's quotients into their own
                    # output ring, stored fp16 from the gpsimd queue itself
                    uvc = sb_pool.tile([N_VIEWS, 2 * BANK], BF16, tag="uvc")
                    act_direct(uvc[:, 0:ZPW], puv[:, 0:ZPW], ACT_FN.Copy)
                    cuv = uvc[:].rearrange(
                        "p (two n) -> p two n", two=2
                    )[:, :, 0:CHUNK]
                    offt = off_pool.tile([N_VIEWS, 2 * CHUNK], F16, tag="g")
                    off3 = offt[:].rearrange("p (two n) -> p two n", two=2)
                    nc.gpsimd.tensor_tensor(off3, cuv, rb, mybir.AluOpType.mult)
                    k = OFF_IDX[p]
                    nc.gpsimd.dma_start(
                        out_off[:, k * 2 * CHUNK : (k + 1) * 2 * CHUNK], offt[:]
                    )
                    continue
                iuv = puv[:].rearrange("p (two n) -> p two n", two=2)[:, :, 0:CHUNK]
                if clamp:
                    tuv = sb_pool.tile([N_VIEWS, 2 * CHUNK], F32, tag="tuv")
                    t3 = tuv[:].rearrange("p (two n) -> p two n", two=2)
                    nc.vector.tensor_tensor(t3, iuv, rb, mybir.AluOpType.mult)
                    nc.vector.tensor_scalar_add(
                        odst[:, 0:1, :], t3[:, 0:1, :], CX
                    )
                    nc.vector.tensor_scalar_add(
                        odst[:, 1:2, :], t3[:, 1:2, :], CY
                    )
                else:
                    nc.vector.tensor_tensor(odst, iuv, rb, mybir.AluOpType.mult)

            # store the pair immediately -- keeps the DMA queues fed;
            # offloaded pairs store only the DVE-computed e=0 half here (the
            # gpsimd half went out fp16 above); the final pairs store per
            # chunk so the tail drains faster
            od = out[:, p * 4 * CHUNK : (p + 1) * 4 * CHUNK]
            od3 = od.rearrange("p (two n) -> p two n", two=2)
            if p in OFF_IDX and not clamp:
                nc.sync.dma_start(od3[:, :, 0:CHUNK], o3[:, :, 0:CHUNK])
            elif p >= PAIRS - 2:
                for e in range(2):
                    nc.sync.dma_start(
                        od3[:, :, e * CHUNK : (e + 1) * CHUNK],
                        o3[:, :, e * CHUNK : (e + 1) * CHUNK],
                    )
            else:
                nc.sync.dma_start(od, otile[:])

    return _install_wait_legalizer(nc)


_module_cache = {}


def _get_module(variant):
    if variant not in _module_cache:
        if variant == "poly":
            _module_cache[variant] = _build_module_poly()
        else:
            _module_cache[variant] = _build_module(variant)
    return _module_cache[variant]


def _kernel_poly(points3d, Rq, t64, f, su, sv, _trace=False):
    import ml_dtypes

    Wu, Wv = _poly_weights(Rq, t64, f, su, sv)       # [PK, V] f64
    Wub = np.ascontiguousarray(Wu.astype(ml_dtypes.bfloat16))
    Wvb = np.ascontiguousarray(Wv.astype(ml_dtypes.bfloat16))

    # monomial matrix from bf16-rounded points (f32 products are near-exact)
    pb = points3d.astype(ml_dtypes.bfloat16).astype(np.float32)  # [N,3]
    pows = []                                        # pows[j][k] = p_j^k
    for j in range(3):
        col = [np.ones(pb.shape[0], np.float32)]
        for _ in range(PDEG):
            col.append(col[-1] * pb[:, j])
        pows.append(col)
    mu = np.empty((PK, pb.shape[0]), np.float32)
    for m, (e1, e2, e3) in enumerate(PMONOS):
        mu[m] = pows[0][e1] * pows[1][e2] * pows[2][e3]
    mu_bf = mu.astype(ml_dtypes.bfloat16)            # [PK, N]

    nc = _get_module("poly")
    in_maps = []
    for c in range(N_CORES):
        mu_core = np.zeros((PK, NPC_PAD), dtype=ml_dtypes.bfloat16)
        mu_core[:, :NPC] = mu_bf[:, c * NPC : (c + 1) * NPC]
        in_maps.append(
            {
                "blob_u": np.ascontiguousarray(
                    np.concatenate([Wub, mu_core], axis=1)
                ),
                "blob_v": np.ascontiguousarray(
                    np.concatenate([Wvb, mu_core], axis=1)
                ),
            }
        )

    res = run_bass_kernel_spmd(
        nc, in_maps, core_ids=list(range(N_CORES)), trace=_trace
    )

    su32 = su.astype(np.float32)[:, None]
    sv32 = sv.astype(np.float32)[:, None]
    full = np.empty((N_VIEWS, N_POINTS, 2), dtype=np.float32)
    for c in range(N_CORES):
        q = res.results[c]["out"].reshape(N_VIEWS, PCHUNKS, 2, PCHUNK)
        qu = q[:, :, 0, :].reshape(N_VIEWS, NPC_PAD)[:, :NPC].astype(np.float32)
        qv = q[:, :, 1, :].reshape(N_VIEWS, NPC_PAD)[:, :NPC].astype(np.float32)
        full[:, c * NPC : (c + 1) * NPC, 0] = CX + su32 * qu
        full[:, c * NPC : (c + 1) * NPC, 1] = CY + sv32 * qv
    if _trace:
        return full, res
    return full


# ---------------------------------------------------------------------------
# Entry point
# ---------------------------------------------------------------------------
def kernel(points3d, euler_angles, translations, focal_length, _trace=False):
    points3d = np.asarray(points3d, dtype=np.float32)
    euler_angles = np.asarray(euler_angles, dtype=np.float32)
    translations = np.asarray(translations, dtype=np.float32)
    focal_length = np.asarray(focal_length, dtype=np.float32)
    f = float(focal_length[0])

    # Is the Z clamp provably inactive?  znega = -(r2.p + tz) >= min_v(-tz -
    # |r2|*max|p|).  The fast paths fold CX/CY into the matmul, which is only
    # exact when no point clamps.
    Rq = _euler_to_matrix(euler_angles.astype(np.float64))
    tz = translations[:, 2].astype(np.float64)
    tx = translations[:, 0].astype(np.float64)
    ty = translations[:, 1].astype(np.float64)
    r2n = np.linalg.norm(Rq[:, 2, :], axis=1)
    pmax = float(np.linalg.norm(points3d.astype(np.float64), axis=1).max())
    znega_lo_v = -tz - r2n * pmax
    znega_lo = float(znega_lo_v.min())
    if znega_lo < max(-Z_MAX * 10.0, 1e-3):
        variant = "clamp"
    else:
        # per-view int8 scales from rigorous bounds (|row(R)| = 1)
        bu = f * (pmax + np.abs(tx)) / znega_lo_v
        bv = f * (pmax + np.abs(ty)) / znega_lo_v
        su = bu * 1.03 / 127.0
        sv = bv * 1.03 / 127.0
        # quantization step too coarse -> prefer the fp16 variant
        variant = "i8" if max(su.max(), sv.max()) < 4.0 else "f16"

    # poly fast path: series residual y^4/(1-y) must be small and the
    # int8 scale fine enough
    if variant == "i8":
        a_v = -tz
        ymax = float((r2n * pmax / a_v).max())
        if a_v.min() > 0.5 and ymax <= 0.33:
            t64 = translations.astype(np.float64)
            return _kernel_poly(
                points3d.astype(np.float64), Rq, t64, f, su, sv, _trace
            )

    if variant == "i8":
        Wu, Wv, Wz = _fold_weights(
            euler_angles, translations, focal_length, "i8", su, sv
        )
    else:
        Wu, Wv, Wz = _fold_weights(euler_angles, translations, focal_length, variant)

    import ml_dtypes

    pT = points3d.T                                   # [3, N] f32
    p_hi = pT.astype(ml_dtypes.bfloat16)              # [3, N]
    p_lo = (pT - p_hi.astype(np.float32)).astype(ml_dtypes.bfloat16)
    ones = np.ones((1, N_POINTS), dtype=ml_dtypes.bfloat16)
    pk = np.concatenate([p_hi, p_lo, p_hi, ones, ones], axis=0)  # [KROWS, N]

    nc = _get_module(variant)
    in_maps = []
    for c in range(N_CORES):
        sl = pk[:, c * NPC : (c + 1) * NPC]
        in_maps.append(
            {
                "blob_u": np.ascontiguousarray(np.concatenate([Wu, sl], axis=1)),
                "blob_v": np.ascontiguousarray(np.concatenate([Wv, sl], axis=1)),
                "blob_z": np.ascontiguousarray(np.concatenate([Wz, sl], axis=1)),
            }
        )

    res = run_bass_kernel_spmd(
        nc, in_maps, core_ids=list(range(N_CORES)), trace=_trace
    )

    full = np.empty((N_VIEWS, N_POINTS, 2), dtype=np.float32)
    for c in range(N_CORES):
        # DRAM layout: 25 pairs x [u(2p) u(2p+1) | v(2p) v(2p+1)]
        q = res.results[c]["out"].reshape(N_VIEWS, PAIRS, 2, 2, CHUNK)
        blk = (
            q.transpose(0, 1, 3, 4, 2)                 # [V, pair, e, n, uv]
            .reshape(N_VIEWS, NPC, 2)
            .astype(np.float32)
        )
        if variant == "i8":
            qo = res.results[c]["out_off"].astype(np.float32)
            qo = qo.reshape(N_VIEWS, len(OFF_PAIRS), 2, CHUNK)
            for k, p in enumerate(OFF_PAIRS):
                cidx = 2 * p + 1
                blk[:, cidx * CHUNK : (cidx + 1) * CHUNK, 0] = qo[:, k, 0, :]
                blk[:, cidx * CHUNK : (cidx + 1) * CHUNK, 1] = qo[:, k, 1, :]
            blk[:, :, 0] = CX + su[:, None].astype(np.float32) * blk[:, :, 0]
            blk[:, :, 1] = CY + sv[:, None].astype(np.float32) * blk[:, :, 1]
        elif variant == "f16":
            qo = res.results[c]["out_off"].astype(np.float32)
            qo = qo.reshape(N_VIEWS, len(OFF_PAIRS), 2, CHUNK)
            for k, p in enumerate(OFF_PAIRS):
                cidx = 2 * p + 1
                blk[:, cidx * CHUNK : (cidx + 1) * CHUNK, 0] = qo[:, k, 0, :]
                blk[:, cidx * CHUNK : (cidx + 1) * CHUNK, 1] = qo[:, k, 1, :]
        full[:, c * NPC : (c + 1) * NPC, :] = blk
    if _trace:
        return full, res
    return full



# revision 36
# speedup vs baseline: 1.0287x; 1.0054x over previous
"""Bundle-adjustment forward projection on 8 Trainium2 NeuronCores.

reference:  R = euler_to_matrix(euler_angles)            [V,3,3]
            pc = einsum('nj,vij->vni', points3d, R) + t  [V,N,3]
            Zc = min(pc_z, -1e-4)
            u = -f*Xc/Zc + CX ; v = f*Yc/Zc + CY         -> [V,N,2]

FAST PATH ("poly"): the projective division is absorbed into the matmul.
With a = -tz > 0 and y = (r2.p)/a bounded (|y| <= ~0.25 for this
problem's geometry), 1/znega = (1/a)(1 + y + y^2 + y^3) + O(y^4), so the
quantized output q_u = (u - CX)/s_u = (f/(s_u a))(r0.p + tx)(1+y+y^2+y^3)
is a polynomial in the point coordinates, truncated to the K=20 monomials
of degree <= 3 (the dropped deg-4 part is < 1.2 px worst-case):
q_u[v,n] = sum_m W_u[m,v] * mu_m(p_n).  The device then does NO
reciprocals and NO elementwise multiplies -- just two K=20 bf16 matmul
streams (u on PE row strip 0 via tile_position (0,0), v on strip 2 via
(64,0), overlapping) plus exactly one PSUM f32 -> SBUF int8 cast per
output element.  Total error ~2.3 px on a ~778 px scale (rel 3.0e-3)
vs the 2e-2 gate.

Poly device pipeline per 512-point chunk: 2 matmuls (u, v) into a
[128,1024] 2-bank PSUM tile (bufs=4 = all 8 banks -- small tiles keep
matmul latency OFF the cast critical path), one FD=1024 cast
(ScalarE activation-Copy or VectorE tensor_copy, both 1x rate on PSUM
f32 src, split 26/23 by measured speed and running CONCURRENTLY on
different banks), one 256 KB int8 store per chunk pair.  The cast pair
is the wall: both engines run ~95% dense for ~28 us.  N is padded
25000 -> 49*512 = 25088 per core.

Measured phase budget (of ~44 us): ~7.5 us fixed NEFF/framework
preamble, ~2.5 us first-input DMA round trip (weights ride in the same
blob as chunk 0, descriptor gen + ~2 us completion receipt), ~29 us
cast-bound steady state, ~4 us tail (last store receipt + drains; the
cast schedule ends on the faster ScalarE).  Things that DON'T work:
loads on the scalar ring (ACT's FIFO stalls ~1.3 us per skinny DMA
trigger and the casts starve); all stores on one ring (~0.85 us/store
ring time saturates it); late stores on the gpsimd ring (the SWDGE
end-drain waits ~3.5 us on its own last receipt); N=1024 matmuls
(2-bank PSUM output -- builds in bass but the NEFF compiler rejects
it); splitting the final chunk's cast/store across engines (regressed).

The legacy reciprocal-based kernel below is kept as a fallback for
inputs where the series gates fail (z-clamp possible, |y| too large, or
int8 scale too coarse).  Its design notes follow:

The kernel quantizes the output: u = CX + s_u[view] * q where q is stored
as int8 and s_u is a per-view scale folded into the matmul weights on the
host (CX = img/2 = 512 cancels exactly: q = (f/s_u)*Xc * (1/znega)).
The worst-case quantization error is ~2 px on a ~770 px scale (~5e-3
relative, gate is 2e-2).  This cuts the HBM store to 6.4 MB/core.  If
the host-side bound makes the scale too coarse (or the Z clamp can
fire), it falls back to an fp16 variant.

Sharding: N=200000 points split across 8 cores (25000 each); every core
computes all V=128 views so the SBUF partition dim is fully used.

PE: u, v, z matmuls live on three separate 32-row groups of the PE array
(tile_position rows 0/32/64) so the three 500-column streams run
concurrently.  Each row group has its own copy of the point columns in
SBUF partitions 0-10 / 32-42 / 64-74.

Pipeline per 500-point chunk: z matmul -> per-chunk ACT Reciprocal into
a deep ring -> u,v matmuls into a 3-buf PSUM ring -> one DVE broadcast
tensor_tensor computing both quotients (fp32 PSUM x recip -> int8 SBUF)
-> per-pair 256 KB store.  Chunk e=1 of every other mid-stream pair is
computed by the GPSIMD engine instead (ACT stages U,V to SBUF bf16, Q7
multiplies into its own ring and stores fp16 via the SWDGE queue),
taking ~11 us off the DVE critical path.

Hard-won scheduling facts baked in here: (1) deep rings everywhere --
compute must never wait on a store's HBM-receipt semaphore (~3.5 us) or
on the 8 DMAHW completion lanes, so the output ring is 12 pairs deep;
(2) input rides TWO DGE paths in parallel (z+v sections on the gpsimd
SWDGE queue, u section on the scalar HWDGE ring which is otherwise
unused) -- a single queue serializes the 1.66 MB input behind ~93 GB/s
and starves the early pipeline; (3) the offloaded chunks' fp16 halves
store from the gpsimd queue itself so the sync store FIFO never waits
on the late Q7 finish.  Engines measure ~47/48/43/38 us busy
(DVE/ACT/PE/GPSIMD) over a ~66 us kernel -- the DVE stream and the ACT
recip+staging stream are the twin walls.

Numerics: inputs ship as a bf16 hi/lo split (K=11 rows against weight
columns [w_hi, w_hi, w_lo, b_hi, b_lo]); only w_lo*p_lo (~2^-18) is
dropped.

Walrus in this build accepts at most ONE semaphore wait per instruction:
TileContext's tail drain is patched to split its waits into one-wait
nops, and a serialized-BIR rewriter injects same-engine NoOps for any
remaining multi-wait instruction.
"""

import numpy as np
from contextlib import ExitStack

import concourse.bass as bass
import concourse.tile as tile
from concourse import mybir
from concourse.bass_utils import run_bass_kernel_spmd
from concourse.vector_clock import ScopedClock, VectorClock

CX = 512.0
CY = 512.0
Z_MAX = -1e-4

N_CORES = 8
N_POINTS = 200000
N_VIEWS = 128
NPC = N_POINTS // N_CORES          # 25000 points per core
CHUNK = 500                        # matmul free dim (fits one PSUM bank)
CHUNKS = NPC // CHUNK              # 50
PAIRS = CHUNKS // 2                # 25
WCOLS = N_VIEWS                    # weight columns per blob section
BLOB = WCOLS + NPC                 # 25128 cols per 11-row section
KROWS = 11
BANK = 512

# chunk e=1 of these pairs runs on the GPSIMD engine instead of the DVE
OFF_PAIRS = (2, 4, 6, 8, 10, 12, 14, 16, 18, 20)
OFF_IDX = {p: i for i, p in enumerate(OFF_PAIRS)}

F32 = mybir.dt.float32
F16 = mybir.dt.float16
I8 = mybir.dt.int8
BF16 = mybir.dt.bfloat16


# ---------------------------------------------------------------------------
# Tile tail-drain workaround: this walrus build only accepts ONE semaphore
# wait per CTRL instruction, but TileContext puts every outstanding proc's
# wait on the single tail Drain.  Emit one-wait nops first instead.
# ---------------------------------------------------------------------------
def _split_drain_and_barrier(self, tick_clock, wait_clock):
    gc = tick_clock.global_clock
    n = len(gc)
    # spread the one-wait nops across all five engine queues so the ~18
    # serial waits (~60 ns each on one sequencer) overlap; the
    # all_engine_barrier below joins them
    engines = [self.nc.sync, self.nc.vector, self.nc.scalar,
               self.nc.tensor, self.nc.gpsimd]
    i = 0
    for p in range(n):
        if gc[p] > 0:
            vec = [0] * n
            vec[p] = gc[p]
            nop = engines[i % len(engines)].nop()
            i += 1
            wait_clock.add_sem_waits(nop.ins, ScopedClock({None: VectorClock(vec)}))
    self.nc.sync.drain()
    self.nc.all_engine_barrier()
    assert self.sems is not None
    popped = self.nc._tile_sem_poison_stack.pop()
    assert popped is self._sem_poison
    self.nc.clear_and_free_semaphores(list(self.sems.allocated().values()))
    self.nc.all_engine_barrier()


tile.TileContext._drain_and_barrier = _split_drain_and_barrier


def _legalize_waits(bir: bytes) -> bytes:
    """This walrus build accepts at most ONE semaphore wait per instruction.
    Split every multi-wait instruction by injecting same-engine NoOps (each
    carrying one wait) immediately before it: engines consume their block
    instructions in order, so the nop's wait completes before the real op."""
    import json as _json

    d = _json.loads(bir)
    ctr = 0
    for f in d["functions"]:
        for b in f["blocks"]:
            newl = []
            for inst in b["instructions"]:
                si = inst.get("sync_info")
                w = (si or {}).get("on_wait") or []
                if len(w) > 1:
                    for extra in w[:-1]:
                        ctr += 1
                        newl.append(
                            {
                                "debug": inst.get("debug", 0),
                                "engine": inst["engine"],
                                "ins": [],
                                "outs": [],
                                "name": f"I-wfix{ctr}",
                                "opcode": "NoOp",
                                "sync_info": {"on_update": [], "on_wait": [extra]},
                            }
                        )
                    si["on_wait"] = [w[-1]]
                newl.append(inst)
            b["instructions"] = newl
    return _json.dumps(d).encode()


def _install_wait_legalizer(nc):
    orig = nc.to_json_bytes

    def to_json_bytes_fixed():
        return _legalize_waits(orig())

    nc.to_json_bytes = to_json_bytes_fixed
    return nc


# ---------------------------------------------------------------------------
# Poly fast path
# ---------------------------------------------------------------------------
PCHUNK = 512                        # poly matmul free dim (= one PSUM bank)
PCHUNKS = 49                        # ceil(25000/512)
NPC_PAD = PCHUNKS * PCHUNK          # 25088 (88 zero-pad points per core)
PDEG = 3                            # monomials kept: the deg-4 part of
                                    # L*(1+y+y^2+y^3) adds < 1.2 px worst-case
# canonical monomial order: all (e1,e2,e3) with e1+e2+e3 <= 3 -> 20 rows
PMONOS = sorted(
    (e1, e2, e3)
    for e1 in range(PDEG + 1)
    for e2 in range(PDEG + 1)
    for e3 in range(PDEG + 1)
    if e1 + e2 + e3 <= PDEG
)
PK = len(PMONOS)                    # 20


def _poly_weights(R, t, f, su, sv):
    """[PK, V] f64 coefficient matrices W_u, W_v of the degree-4 series.

    q_u[v,n] = sum_m W_u[m,v] * mu_m(p_n), where
    q_u = (f/(su*a)) * (r0.p + tx) * (1 + y + y^2 + y^3),  y = (r2.p)/a.
    """
    V = R.shape[0]
    r0, r1, r2 = R[:, 0, :], R[:, 1, :], R[:, 2, :]
    tx, ty, tz = t[:, 0], t[:, 1], t[:, 2]
    a = -tz

    def pmul(A, B):
        out = {}
        for ea, ca in A.items():
            for eb, cb in B.items():
                k = (ea[0] + eb[0], ea[1] + eb[1], ea[2] + eb[2])
                out[k] = out.get(k, 0) + ca * cb
        return out

    def padd(A, B):
        out = dict(A)
        for k, c in B.items():
            out[k] = out.get(k, 0) + c
        return out

    g = r2 / a[:, None]
    Y = {(1, 0, 0): g[:, 0], (0, 1, 0): g[:, 1], (0, 0, 1): g[:, 2]}
    Y2 = pmul(Y, Y)
    Y3 = pmul(Y2, Y)
    S = padd(padd({(0, 0, 0): np.ones(V)}, Y), padd(Y2, Y3))
    Lu = {(0, 0, 0): tx, (1, 0, 0): r0[:, 0], (0, 1, 0): r0[:, 1], (0, 0, 1): r0[:, 2]}
    Lv = {(0, 0, 0): ty, (1, 0, 0): r1[:, 0], (0, 1, 0): r1[:, 1], (0, 0, 1): r1[:, 2]}
    au = f / (su * a)
    av = -f / (sv * a)
    Pu = {k: au * c for k, c in pmul(Lu, S).items()}
    Pv = {k: av * c for k, c in pmul(Lv, S).items()}
    Wu = np.stack([Pu.get(m, np.zeros(V)) for m in PMONOS], 0)
    Wv = np.stack([Pv.get(m, np.zeros(V)) for m in PMONOS], 0)
    return Wu, Wv


def _poly_cast_schedule():
    """Per-chunk cast engine: greedy least-loaded with measured per-op costs
    (ACT (1024+202)/1.2 ns, DVE tensor_copy (1024+135)/0.96 ns)."""
    tA = tD = 0.0
    cA, cD = 1022.0, 1161.0
    eng = []
    for _ in range(PCHUNKS):
        if tA + cA <= tD + cD:
            eng.append("A")
            tA += cA
        else:
            eng.append("D")
            tD += cD
    # end on the faster engine: the final cast gates the tail (store +
    # receipt + drain), so swap the last D with the nearest preceding A
    if eng[-1] == "D":
        i = len(eng) - 1 - eng[::-1].index("A")
        eng[i], eng[-1] = "D", "A"
    return eng


def _build_module_poly():
    WCOL = N_VIEWS                  # weight columns prepended to each blob
    BLOBW = WCOL + NPC_PAD
    nc = bass.Bass()
    # blobs: [W (20x128) | monomials (20x25088)] -- embedding the weights
    # in the same tensor lets ONE first-piece DMA deliver both the lhsT and
    # chunk 0, removing a separate weight-DMA from the ramp critical path
    bu_d = nc.declare_dram_parameter("blob_u", [PK, BLOBW], BF16, isOutput=False)
    bv_d = nc.declare_dram_parameter("blob_v", [PK, BLOBW], BF16, isOutput=False)
    out_d = nc.declare_dram_parameter("out", [N_VIEWS, 2 * NPC_PAD], I8, isOutput=True)

    with tile.TileContext(nc) as tc, ExitStack() as ctx:
        const_pool = ctx.enter_context(tc.tile_pool(name="const", bufs=1))
        # one [128, 1024] f32 tile = 2 PSUM banks; 4 bufs = all 8 banks.
        # Small tiles keep matmul latency off the cast critical path.
        psum_pool = ctx.enter_context(tc.tile_pool(name="ps", bufs=4, space="PSUM"))
        out_pool = ctx.enter_context(tc.tile_pool(name="out", bufs=14))

        # blob tiles: u copy at partitions 0-19, v copy at 64-83
        big = const_pool.tile([64 + PK, BLOBW], BF16, tag="mu")

        # warm the ACT spline tables before any input lands
        warm = const_pool.tile([1, 2], F32, tag="warm")
        nc.vector.memset(warm[:], 1.0)
        nc.scalar.copy(warm[0:1, 1:2], warm[0:1, 0:1])

        # Input loads must NOT ride the scalar ring: the ACT engine FIFO
        # stalls behind each skinny-partition DMA trigger (~1.3 us apiece)
        # and the casts starve.  u blob on the sync HWDGE ring, v blob on
        # the gpsimd SWDGE ring; ACT issues nothing.  Geometric piece
        # sizes: the tiny first piece (weights + chunk 0) gates the first
        # matmul on ~26 KB instead of ~100 KB.
        edges = [0, WCOL + 512, WCOL + 1536, WCOL + 3072, WCOL + 5120,
                 WCOL + 7680, WCOL + 10752, WCOL + 14336, WCOL + 17920,
                 WCOL + 21504, BLOBW]
        for i in range(len(edges) - 1):
            lo, hi = edges[i], edges[i + 1]
            nc.sync.dma_start(big[0:PK, lo:hi], bu_d[:, lo:hi])
            nc.gpsimd.dma_start(big[64 : 64 + PK, lo:hi], bv_d[:, lo:hi])

        lhs_u = big[0:PK, 0:WCOL]
        lhs_v = big[64 : 64 + PK, 0:WCOL]
        mu_u = big[0:PK, WCOL:]
        mu_v = big[64 : 64 + PK, WCOL:]
        eng = _poly_cast_schedule()

        otile = None
        for c in range(PCHUNKS):
            csl = slice(c * PCHUNK, (c + 1) * PCHUNK)
            puv = psum_pool.tile([N_VIEWS, 2 * PCHUNK], F32, tag="puv")
            nc.tensor.matmul(
                puv[:, 0:PCHUNK], lhs_u, mu_u[:, csl], tile_position=(0, 0)
            )
            nc.tensor.matmul(
                puv[:, PCHUNK : 2 * PCHUNK], lhs_v, mu_v[:, csl],
                tile_position=(64, 0),
            )
            half = c % 2
            if half == 0:
                otile = out_pool.tile([N_VIEWS, 4 * PCHUNK], I8, tag="o")
            dst = otile[:, half * 2 * PCHUNK : (half + 1) * 2 * PCHUNK]
            if eng[c] == "A":
                nc.scalar.copy(dst, puv[:])
            else:
                nc.vector.tensor_copy(dst, puv[:])
            # stores split across the sync and gpsimd rings (one ring can't
            # carry loads + all 25 stores), but the LAST gpsimd store must
            # land well before the end: the SWDGE end-of-kernel drain waits
            # on its own last receipt (~3.5 us when a store issues at ~40 us)
            if c >= PCHUNKS - 5:
                # tail: per-chunk stores, sync ring only
                nc.sync.dma_start(
                    out_d[:, c * 2 * PCHUNK : (c + 1) * 2 * PCHUNK], dst
                )
            elif half == 1:
                W = (half + 1) * 2 * PCHUNK
                q = nc.sync if (c // 2) % 2 == 0 else nc.gpsimd
                q.dma_start(
                    out_d[:, (c - half) * 2 * PCHUNK : (c - half) * 2 * PCHUNK + W],
                    otile[:, 0:W],
                )

    return _install_wait_legalizer(nc)


# ---------------------------------------------------------------------------
# Host-side math
# ---------------------------------------------------------------------------
def _euler_to_matrix(e):
    """[V,3] -> [V,3,3], Rx @ Ry @ Rz (same convention as the reference)."""
    x, y, z = e[:, 0], e[:, 1], e[:, 2]
    c1, s1 = np.cos(x), np.sin(x)
    c2, s2 = np.cos(y), np.sin(y)
    c3, s3 = np.cos(z), np.sin(z)
    zero = np.zeros_like(x)
    one = np.ones_like(x)
    Rx = np.stack([one, zero, zero, zero, c1, -s1, zero, s1, c1], -1).reshape(-1, 3, 3)
    Ry = np.stack([c2, zero, s2, zero, one, zero, -s2, zero, c2], -1).reshape(-1, 3, 3)
    Rz = np.stack([c3, -s3, zero, s3, c3, zero, zero, zero, one], -1).reshape(-1, 3, 3)
    return Rx @ Ry @ Rz


def _pack(w, b):
    # -> [KROWS, V] bf16 lhsT: cols per view = [w_hi(3), w_hi(3), w_lo(3),
    # b_hi, b_lo] matching point rows [p_hi(3), p_lo(3), p_hi(3), 1, 1]
    import ml_dtypes

    w_hi = w.astype(ml_dtypes.bfloat16)
    w_lo = (w - w_hi.astype(np.float64)).astype(ml_dtypes.bfloat16)
    b_hi = b.astype(ml_dtypes.bfloat16)
    b_lo = (b - b_hi.astype(np.float64)).astype(ml_dtypes.bfloat16)
    return np.concatenate(
        [w_hi.T, w_hi.T, w_lo.T, b_hi[None, :], b_lo[None, :]], axis=0
    )


def _fold_weights(euler_angles, translations, focal_length, variant, su=None, sv=None):
    """Build the three [KROWS, V] stationary matrices."""
    R = _euler_to_matrix(euler_angles.astype(np.float64))
    t = translations.astype(np.float64)
    f = float(focal_length[0])
    r0, r1, r2 = R[:, 0, :], R[:, 1, :], R[:, 2, :]
    tx, ty, tz = t[:, 0], t[:, 1], t[:, 2]

    if variant == "i8":
        # q_u = (f/s_u)*Xc/znega : the CX fold cancels exactly (CX=512)
        wU = (f / su)[:, None] * r0
        bU = (f / su) * tx
        wV = (-f / sv)[:, None] * r1
        bV = (-f / sv) * ty
    elif variant == "clamp":
        # numerators without the CX/CY fold (added on DVE after the division)
        wU = f * r0
        bU = f * tx
        wV = -f * r1
        bV = -f * ty
    else:  # f16
        wU = f * r0 - CX * r2
        bU = f * tx - CX * tz
        wV = -f * r1 - CY * r2
        bV = -f * ty - CY * tz
    wZ = -r2
    bZ = -tz
    return _pack(wU, bU), _pack(wV, bV), _pack(wZ, bZ)


# ---------------------------------------------------------------------------
# Bass module
# ---------------------------------------------------------------------------
def _build_module(variant):
    i8 = variant == "i8"
    clamp = variant == "clamp"
    ODT = I8 if i8 else F16
    off_pairs = OFF_PAIRS if not clamp else ()

    nc = bass.Bass()
    blob_u = nc.declare_dram_parameter("blob_u", [KROWS, BLOB], BF16, isOutput=False)
    blob_v = nc.declare_dram_parameter("blob_v", [KROWS, BLOB], BF16, isOutput=False)
    blob_z = nc.declare_dram_parameter("blob_z", [KROWS, BLOB], BF16, isOutput=False)
    out = nc.declare_dram_parameter("out", [N_VIEWS, 2 * NPC], ODT, isOutput=True)
    if off_pairs:
        out_off = nc.declare_dram_parameter(
            "out_off", [N_VIEWS, len(off_pairs) * 2 * CHUNK], F16, isOutput=True
        )

    with tile.TileContext(nc) as tc, ExitStack() as ctx:
        const_pool = ctx.enter_context(tc.tile_pool(name="const", bufs=1))
        # PSUM: U,V ring gets 3 bufs (6 banks) so the ACT staging copy of an
        # offloaded chunk is never inside the ring's reuse window; z tiles
        # are per-chunk single banks (2 bufs = 2 banks).  6 + 2 = 8 banks.
        psum_uv = ctx.enter_context(tc.tile_pool(name="psuv", bufs=3, space="PSUM"))
        psum_z = ctx.enter_context(tc.tile_pool(name="psz", bufs=2, space="PSUM"))
        # per-chunk reciprocals in a deep ring: the gpsimd consumer finishes
        # ~5us after its pair starts and must not block recip reuse
        recip_pool = ctx.enter_context(tc.tile_pool(name="recip", bufs=12))
        sb_pool = ctx.enter_context(tc.tile_pool(name="sb", bufs=6))
        # deep output ring: a pair's compute must never wait on the
        # data-landed semaphore of a recent store
        out_pool = ctx.enter_context(tc.tile_pool(name="out", bufs=12))
        # separate ring for the gpsimd-computed chunks: their later finish
        # must not block the sync store queue or ring A
        off_pool = ctx.enter_context(tc.tile_pool(name="off", bufs=6))

        # blob sections land at partition rows 0-10 (u), 32-42 (v), 64-74 (z)
        # so each PE row group streams its own rhs copy.
        btile = const_pool.tile([64 + KROWS, BLOB], BF16, tag="blob")
        # z first: the recip chain (z matmul -> ACT -> DVE) gates chunk 0
        sections = ((64, blob_z), (0, blob_u), (32, blob_v))

        # Tiny weights+first-pair pieces on separate queues so their issue
        # and completion don't serialize; then 12 pieces up-front on the
        # gpsimd (SWDGE) queue (kept under the SWDGE ring depth so no
        # mid-stream drain blocks late pieces), interleaved across sections
        # in consumption order.
        SPLIT = WCOLS + 2 * CHUNK
        for (base, blob), eng in zip(sections, (nc.sync, nc.scalar, nc.sync)):
            eng.dma_start(btile[base : base + KROWS, 0:SPLIT], blob[:, 0:SPLIT])
        GRPS = (6, 10, 16, 16)
        edges = [2]
        for g in GRPS:
            edges.append(edges[-1] + g)
        # u-section pieces ride the scalar HWDGE queue (qACT ring, otherwise
        # unused): input lands via two independent DGE paths (~2x faster than
        # one SWDGE queue).  Only 2 u pieces so the issue slots ahead of the
        # first Reciprocal on the ACT queue stay short.
        for lo_c, hi_c in ((2, 18), (18, 50)):
            nc.scalar.dma_start(
                btile[0:KROWS, WCOLS + lo_c * CHUNK : WCOLS + hi_c * CHUNK],
                blob_u[:, WCOLS + lo_c * CHUNK : WCOLS + hi_c * CHUNK],
            )
        for gi in range(len(GRPS)):
            for base, blob in sections:
                if base == 0:
                    continue
                lo = WCOLS + edges[gi] * CHUNK
                hi = WCOLS + edges[gi + 1] * CHUNK
                nc.gpsimd.dma_start(
                    btile[base : base + KROWS, lo:hi], blob[:, lo:hi]
                )

        ACT_FN = mybir.ActivationFunctionType

        def act_direct(out_ap, in_ap, func, bias=0.0, scale=1.0, alpha=0.0):
            # same lowering as nc.scalar.activation but without the
            # Reciprocal accuracy guard (measured 1.2e-5 rel err on our
            # [1.1, 3.6] domain, far inside the output tolerance)
            eng = nc.scalar
            ins = [eng.lower_ap(in_ap)]
            for val in (bias, scale, alpha):
                ins.append(mybir.ImmediateValue(dtype=mybir.dt.float32, value=val))
            return eng.add_instruction(
                mybir.InstActivation(
                    name=nc.get_next_instruction_name(),
                    func=func,
                    ins=ins,
                    outs=[eng.lower_ap(out_ap)],
                )
            )

        # pre-warm the ACT spline tables (~2.7 us) under the input transfer
        warm = sb_pool.tile([1, 2], F32, tag="warm")
        nc.vector.memset(warm[:], 1.0)
        act_direct(warm[0:1, 1:2], warm[0:1, 0:1], ACT_FN.Reciprocal)

        ZPW = 2 * CHUNK + (BANK - CHUNK)   # 1012
        lhs_u = btile[0:KROWS, 0:WCOLS]
        lhs_v = btile[32 : 32 + KROWS, 0:WCOLS]
        lhs_z = btile[64 : 64 + KROWS, 0:WCOLS]

        def make_recip(c):
            # z matmul + per-chunk Reciprocal into the deep recip ring
            pz = psum_z.tile([N_VIEWS, BANK], F32, tag="pz")
            rhs_z = btile[
                64 : 64 + KROWS, WCOLS + c * CHUNK : WCOLS + (c + 1) * CHUNK
            ]
            nc.tensor.matmul(pz[:, 0:CHUNK], lhs_z, rhs_z, tile_position=(64, 0))
            rtile = recip_pool.tile([N_VIEWS, BANK], BF16, tag="r")
            if clamp:
                zcl = sb_pool.tile([N_VIEWS, BANK], F32, tag="zcl")
                nc.vector.tensor_scalar_max(zcl[:, 0:CHUNK], pz[:, 0:CHUNK], -Z_MAX)
                act_direct(rtile[:, 0:CHUNK], zcl[:, 0:CHUNK], ACT_FN.Reciprocal)
            else:
                act_direct(rtile[:, 0:CHUNK], pz[:, 0:CHUNK], ACT_FN.Reciprocal)
            return rtile

        for p in range(PAIRS):
            c0 = 2 * p
            rtiles = [make_recip(c0), make_recip(c0 + 1)]

            # output tile: planar per pair [u(2p) u(2p+1) | v(2p) v(2p+1)]
            otile = out_pool.tile([N_VIEWS, 4 * CHUNK], ODT, tag="o")
            o3 = otile[:].rearrange("p (two n) -> p two n", two=2)

            for e in range(2):
                c = c0 + e
                puv = psum_uv.tile([N_VIEWS, 2 * BANK], F32, tag="puv")
                rhs = btile[0:KROWS, WCOLS + c * CHUNK : WCOLS + (c + 1) * CHUNK]
                rhs_v = btile[
                    32 : 32 + KROWS, WCOLS + c * CHUNK : WCOLS + (c + 1) * CHUNK
                ]
                nc.tensor.matmul(
                    puv[:, 0:CHUNK], lhs_u, rhs, tile_position=(0, 0)
                )
                nc.tensor.matmul(
                    puv[:, BANK : BANK + CHUNK], lhs_v, rhs_v,
                    tile_position=(32, 0),
                )

                rb = (
                    rtiles[e][:, 0:CHUNK]
                    .unsqueeze(1)
                    .broadcast_to([N_VIEWS, 2, CHUNK])
                )
                odst = o3[:, :, e * CHUNK : (e + 1) * CHUNK]
                if e == 1 and p in OFF_IDX and not clamp:
                    # stage U,V to SBUF (frees the PSUM slot early) and let
                    # the Q7 cores do this chunk's quotients into their own
                    # output ring, stored fp16 from the gpsimd queue itself
                    uvc = sb_pool.tile([N_VIEWS, 2 * BANK], BF16, tag="uvc")
                    act_direct(uvc[:, 0:ZPW], puv[:, 0:ZPW], ACT_FN.Copy)
                    cuv = uvc[:].rearrange(
                        "p (two n) -> p two n", two=2
                    )[:, :, 0:CHUNK]
                    offt = off_pool.tile([N_VIEWS, 2 * CHUNK], F16, tag="g")
                    off3 = offt[:].rearrange("p (two n) -> p two n", two=2)
                    nc.gpsimd.tensor_tensor(off3, cuv, rb, mybir.AluOpType.mult)
                    k = OFF_IDX[p]
                    nc.gpsimd.dma_start(
                        out_off[:, k * 2 * CHUNK : (k + 1) * 2 * CHUNK], offt[:]
                    )
                    continue
                iuv = puv[:].rearrange("p (two n) -> p two n", two=2)[:, :, 0:CHUNK]
                if clamp:
                    tuv = sb_pool.tile([N_VIEWS, 2 * CHUNK], F32, tag="tuv")
                    t3 = tuv[:].rearrange("p (two n) -> p two n", two=2)
                    nc.vector.tensor_tensor(t3, iuv, rb, mybir.AluOpType.mult)
                    nc.vector.tensor_scalar_add(
                        odst[:, 0:1, :], t3[:, 0:1, :], CX
                    )
                    nc.vector.tensor_scalar_add(
                        odst[:, 1:2, :], t3[:, 1:2, :], CY
                    )
                else:
                    nc.vector.tensor_tensor(odst, iuv, rb, mybir.AluOpType.mult)

            # store the pair immediately -- keeps the DMA queues fed;
            # offloaded pairs store only the DVE-computed e=0 half here (the
            # gpsimd half went out fp16 above); the final pairs store per
            # chunk so the tail drains faster
            od = out[:, p * 4 * CHUNK : (p + 1) * 4 * CHUNK]
            od3 = od.rearrange("p (two n) -> p two n", two=2)
            if p in OFF_IDX and not clamp:
                nc.sync.dma_start(od3[:, :, 0:CHUNK], o3[:, :, 0:CHUNK])
            elif p >= PAIRS - 2:
                for e in range(2):
                    nc.sync.dma_start(
                        od3[:, :, e * CHUNK : (e + 1) * CHUNK],
                        o3[:, :, e * CHUNK : (e + 1) * CHUNK],
                    )
            else:
                nc.sync.dma_start(od, otile[:])

    return _install_wait_legalizer(nc)


_module_cache = {}


def _get_module(variant):
    if variant not in _module_cache:
        if variant == "poly":
            _module_cache[variant] = _build_module_poly()
        else:
            _module_cache[variant] = _build_module(variant)
    return _module_cache[variant]


def _kernel_poly(points3d, Rq, t64, f, su, sv, _trace=False):
    import ml_dtypes

    Wu, Wv = _poly_weights(Rq, t64, f, su, sv)       # [PK, V] f64
    Wub = np.ascontiguousarray(Wu.astype(ml_dtypes.bfloat16))
    Wvb = np.ascontiguousarray(Wv.astype(ml_dtypes.bfloat16))

    # monomial matrix from bf16-rounded points (f32 products are near-exact)
    pb = points3d.astype(ml_dtypes.bfloat16).astype(np.float32)  # [N,3]
    pows = []                                        # pows[j][k] = p_j^k
    for j in range(3):
        col = [np.ones(pb.shape[0], np.float32)]
        for _ in range(PDEG):
            col.append(col[-1] * pb[:, j])
        pows.append(col)
    mu = np.empty((PK, pb.shape[0]), np.float32)
    for m, (e1, e2, e3) in enumerate(PMONOS):
        mu[m] = pows[0][e1] * pows[1][e2] * pows[2][e3]
    mu_bf = mu.astype(ml_dtypes.bfloat16)            # [PK, N]

    nc = _get_module("poly")
    in_maps = []
    for c in range(N_CORES):
        mu_core = np.zeros((PK, NPC_PAD), dtype=ml_dtypes.bfloat16)
        mu_core[:, :NPC] = mu_bf[:, c * NPC : (c + 1) * NPC]
        in_maps.append(
            {
                "blob_u": np.ascontiguousarray(
                    np.concatenate([Wub, mu_core], axis=1)
                ),
                "blob_v": np.ascontiguousarray(
                    np.concatenate([Wvb, mu_core], axis=1)
                ),
            }
        )

    res = run_bass_kernel_spmd(
        nc, in_maps, core_ids=list(range(N_CORES)), trace=_trace
    )

    su32 = su.astype(np.float32)[:, None]
    sv32 = sv.astype(np.float32)[:, None]
    full = np.empty((N_VIEWS, N_POINTS, 2), dtype=np.float32)
    for c in range(N_CORES):
        q = res.results[c]["out"].reshape(N_VIEWS, PCHUNKS, 2, PCHUNK)
        qu = q[:, :, 0, :].reshape(N_VIEWS, NPC_PAD)[:, :NPC].astype(np.float32)
        qv = q[:, :, 1, :].reshape(N_VIEWS, NPC_PAD)[:, :NPC].astype(np.float32)
        full[:, c * NPC : (c + 1) * NPC, 0] = CX + su32 * qu
        full[:, c * NPC : (c + 1) * NPC, 1] = CY + sv32 * qv
    if _trace:
        return full, res
    return full


# ---------------------------------------------------------------------------
# Entry point
# ---------------------------------------------------------------------------
def kernel(points3d, euler_angles, translations, focal_length, _trace=False):
    points3d = np.asarray(points3d, dtype=np.float32)
    euler_angles = np.asarray(euler_angles, dtype=np.float32)
    translations = np.asarray(translations, dtype=np.float32)
    focal_length = np.asarray(focal_length, dtype=np.float32)
    f = float(focal_length[0])

    # Is the Z clamp provably inactive?  znega = -(r2.p + tz) >= min_v(-tz -
    # |r2|*max|p|).  The fast paths fold CX/CY into the matmul, which is only
    # exact when no point clamps.
    Rq = _euler_to_matrix(euler_angles.astype(np.float64))
    tz = translations[:, 2].astype(np.float64)
    tx = translations[:, 0].astype(np.float64)
    ty = translations[:, 1].astype(np.float64)
    r2n = np.linalg.norm(Rq[:, 2, :], axis=1)
    pmax = float(np.linalg.norm(points3d.astype(np.float64), axis=1).max())
    znega_lo_v = -tz - r2n * pmax
    znega_lo = float(znega_lo_v.min())
    if znega_lo < max(-Z_MAX * 10.0, 1e-3):
        variant = "clamp"
    else:
        # per-view int8 scales from rigorous bounds (|row(R)| = 1)
        bu = f * (pmax + np.abs(tx)) / znega_lo_v
        bv = f * (pmax + np.abs(ty)) / znega_lo_v
        su = bu * 1.03 / 127.0
        sv = bv * 1.03 / 127.0
        # quantization step too coarse -> prefer the fp16 variant
        variant = "i8" if max(su.max(), sv.max()) < 4.0 else "f16"

    # poly fast path: series residual y^4/(1-y) must be small and the
    # int8 scale fine enough
    if variant == "i8":
        a_v = -tz
        ymax = float((r2n * pmax / a_v).max())
        if a_v.min() > 0.5 and ymax <= 0.33:
            t64 = translations.astype(np.float64)
            return _kernel_poly(
                points3d.astype(np.float64), Rq, t64, f, su, sv, _trace
            )

    if variant == "i8":
        Wu, Wv, Wz = _fold_weights(
            euler_angles, translations, focal_length, "i8", su, sv
        )
    else:
        Wu, Wv, Wz = _fold_weights(euler_angles, translations, focal_length, variant)

    import ml_dtypes

    pT = points3d.T                                   # [3, N] f32
    p_hi = pT.astype(ml_dtypes.bfloat16)              # [3, N]
    p_lo = (pT - p_hi.astype(np.float32)).astype(ml_dtypes.bfloat16)
    ones = np.ones((1, N_POINTS), dtype=ml_dtypes.bfloat16)
    pk = np.concatenate([p_hi, p_lo, p_hi, ones, ones], axis=0)  # [KROWS, N]

    nc = _get_module(variant)
    in_maps = []
    for c in range(N_CORES):
        sl = pk[:, c * NPC : (c + 1) * NPC]
        in_maps.append(
            {
                "blob_u": np.ascontiguousarray(np.concatenate([Wu, sl], axis=1)),
                "blob_v": np.ascontiguousarray(np.concatenate([Wv, sl], axis=1)),
                "blob_z": np.ascontiguousarray(np.concatenate([Wz, sl], axis=1)),
            }
        )

    res = run_bass_kernel_spmd(
        nc, in_maps, core_ids=list(range(N_CORES)), trace=_trace
    )

    full = np.empty((N_VIEWS, N_POINTS, 2), dtype=np.float32)
    for c in range(N_CORES):
        # DRAM layout: 25 pairs x [u(2p) u(2p+1) | v(2p) v(2p+1)]
        q = res.results[c]["out"].reshape(N_VIEWS, PAIRS, 2, 2, CHUNK)
        blk = (
            q.transpose(0, 1, 3, 4, 2)                 # [V, pair, e, n, uv]
            .reshape(N_VIEWS, NPC, 2)
            .astype(np.float32)
        )
        if variant == "i8":
            qo = res.results[c]["out_off"].astype(np.float32)
            qo = qo.reshape(N_VIEWS, len(OFF_PAIRS), 2, CHUNK)
            for k, p in enumerate(OFF_PAIRS):
                cidx = 2 * p + 1
                blk[:, cidx * CHUNK : (cidx + 1) * CHUNK, 0] = qo[:, k, 0, :]
                blk[:, cidx * CHUNK : (cidx + 1) * CHUNK, 1] = qo[:, k, 1, :]
            blk[:, :, 0] = CX + su[:, None].astype(np.float32) * blk[:, :, 0]
            blk[:, :, 1] = CY + sv[:, None].astype(np.float32) * blk[:, :, 1]
        elif variant == "f16":
            qo = res.results[c]["out_off"].astype(np.float32)
            qo = qo.reshape(N_VIEWS, len(OFF_PAIRS), 2, CHUNK)
            for k, p in enumerate(OFF_PAIRS):
                cidx = 2 * p + 1
                blk[:, cidx * CHUNK : (cidx + 1) * CHUNK, 0] = qo[:, k, 0, :]
                blk[:, cidx * CHUNK : (cidx + 1) * CHUNK, 1] = qo[:, k, 1, :]
        full[:, c * NPC : (c + 1) * NPC, :] = blk
    if _trace:
        return full, res
    return full

